# revision 9
# baseline (speedup 1.0000x reference)
"""AMMLinear (VQ codebook) forward on 8 TRN2 NeuronCores.

The straight-through estimator makes the forward VALUE exactly
    out[n, o] = sum_c lut_dq[c, argmin_k dist(x_cn, cent_ck), o] + bias[o]
with lut = centroids @ weight (per codebook) and lut_dq a global-scale int8
quantize-dequantize of lut.  The softmax/attention path only shapes gradients.

Sharding: tokens (BN=4096 -> 512/core) for the score/argmin phase, output
features (4096 -> 512/core) for the lut/gather phase.  One AllGather moves the
bf16 argmin indices (64KB/core), one AllReduce-max the quantization scale.
Every core then expands all 4096 tokens' one-hot codes locally (replication
matmul + is_equal) and computes its o-shard of the gather matmul.
Output is assembled host-side by concatenating the per-core o-shards.
"""

import numpy as np

import concourse.bass as bass
import concourse.mybir as mybir
import concourse.tile as tile
import concourse.bass_isa as bass_isa
from concourse import bacc
from concourse.bass_utils import run_bass_kernel_spmd
from concourse.masks import make_identity

F32 = mybir.dt.float32
F32R = mybir.dt.float32r
BF16 = mybir.dt.bfloat16

N_CORES = 8
NC, K, IN_F, OUT_F = 64, 16, 4096, 4096
SUBV = IN_F // NC          # 64
BN = 4096                  # 2*2048 tokens
TOK = BN // N_CORES        # 512 tokens per core
NT = TOK // 128            # 4 token tiles per core
NPAIR = NC // 2            # 32 codebook pairs
CK = NC * K                # 1024 (codebook,centroid) flat index
NCHUNK = CK // 128         # 8 contraction chunks
OSH = OUT_F // N_CORES     # 512 out features per core
MAGIC = 12582912.0         # 1.5 * 2**23: fp32 round-to-nearest-even trick
BIG = 1024.0

_CACHE = {}


def _build():
    nc = bacc.Bacc("TRN2", target_bir_lowering=False, debug=False,
                   num_devices=N_CORES)

    xT = nc.declare_dram_parameter("xT", [IN_F, TOK], F32, isOutput=False)
    cbd = nc.declare_dram_parameter("cbd", [128, CK], F32, isOutput=False)
    c2r = nc.declare_dram_parameter("c2r", [128, CK], F32, isOutput=False)
    iotar = nc.declare_dram_parameter("iotar", [128, CK], F32, isOutput=False)
    wsh_h = nc.declare_dram_parameter("wsh_h", [IN_F, OSH], BF16, isOutput=False)
    wsh_l = nc.declare_dram_parameter("wsh_l", [IN_F, OSH], BF16, isOutput=False)
    cbd_h = nc.declare_dram_parameter("cbd_h", [128, CK], BF16, isOutput=False)
    cbd_l = nc.declare_dram_parameter("cbd_l", [128, CK], BF16, isOutput=False)
    biasr = nc.declare_dram_parameter("biasr", [128, OSH], F32, isOutput=False)
    emat = nc.declare_dram_parameter("emat", [NC, CK], BF16, isOutput=False)
    kcol = nc.declare_dram_parameter("kcol", [128, 1], F32, isOutput=False)
    out = nc.declare_dram_parameter("out", [BN, OSH], BF16, isOutput=True)

    with tile.TileContext(nc) as tc:
        with (
            tc.tile_pool(name="consts", bufs=1) as constp,
            tc.tile_pool(name="xt", bufs=8) as xtp,
            tc.tile_pool(name="wt", bufs=6) as wp,
            tc.tile_pool(name="xct", bufs=3) as xctp,
            tc.tile_pool(name="work", bufs=2) as workp,
            tc.tile_pool(name="stg", bufs=16) as stgp,
            tc.tile_pool(name="lut", bufs=1) as lutp,
            tc.tile_pool(name="outs", bufs=4) as outp,
            tc.tile_pool(name="ps", bufs=8, space="PSUM") as psp,
            tc.tile_pool(name="dram", bufs=1, space="DRAM") as dramp,
        ):
            # ---- constants -------------------------------------------------
            cbd_sb = constp.tile([128, CK], F32, tag="cbd_sb")
            nc.sync.dma_start(out=cbd_sb[:, :], in_=cbd.ap()[:, :])
            c2_sb = constp.tile([128, CK], F32, tag="c2_sb")
            nc.sync.dma_start(out=c2_sb[:, :], in_=c2r.ap()[:, :])
            iota_sb = constp.tile([128, CK], F32, tag="iota_sb")
            nc.gpsimd.dma_start(out=iota_sb[:, :], in_=iotar.ap()[:, :])
            bias_sb = constp.tile([128, OSH], F32, tag="bias_sb")
            nc.gpsimd.dma_start(out=bias_sb[:, :], in_=biasr.ap()[:, :])
            emat_sb = constp.tile([NC, CK], BF16, tag="emat_sb")
            nc.gpsimd.dma_start(out=emat_sb[:, :], in_=emat.ap()[:, :])
            kcol_sb = constp.tile([128, 1], F32, tag="kcol_sb")
            nc.gpsimd.dma_start(out=kcol_sb[:, :], in_=kcol.ap()[:, :])
            identb = constp.tile([128, 128], BF16, tag="identb")
            make_identity(nc, identb[:, :])
            identf = constp.tile([128, 128], F32, tag="identf")
            make_identity(nc, identf[:, :])
            # bf16 hi/lo centroid blockdiag for the 3-pass lut matmul
            cbdh_sb = constp.tile([128, CK], BF16, tag="cbdh_sb")
            nc.scalar.dma_start(out=cbdh_sb[:, :], in_=cbd_h.ap()[:, :])
            cbdl_sb = constp.tile([128, CK], BF16, tag="cbdl_sb")
            nc.scalar.dma_start(out=cbdl_sb[:, :], in_=cbd_l.ap()[:, :])

            # DRAM scratch for collectives
            kt_bounce = dramp.tile([NC, TOK], BF16, tag="kt_bounce")
            kt_all = dramp.tile([N_CORES * NC, TOK], BF16, tag="kt_all")
            mx_in = dramp.tile([1, 16], F32, tag="mx_in")
            mx_out = dramp.tile([1, 16], F32, tag="mx_out")

            # ---- phase 1: scores (transposed), argmin, kmin^T --------------
            # xc^T[ck, n] via cent-stationary fp32 matmuls (4 pairs col-packed
            # per 128-row chunk), PE-transposed back to score[n, ck].
            s_tiles = [workp.tile([128, CK], F32, tag=f"score{t}",
                                  name=f"score{t}") for t in range(NT)]
            for j in range(NCHUNK):
                ps_xct = psp.tile([128, TOK], F32, tag="ps")
                for mcol in range(4):
                    p = 4 * j + mcol
                    xt_t = xtp.tile([128, TOK], F32, tag="xt")
                    nc.sync.dma_start(out=xt_t[:, :],
                                      in_=xT.ap()[128 * p:128 * (p + 1), :])
                    nc.tensor.matmul(
                        ps_xct[32 * mcol:32 * (mcol + 1), :],
                        lhsT=cbd_sb[:, 32 * p:32 * (p + 1)],
                        rhs=xt_t[:, :],
                        start=True, stop=True,
                        tile_position=(0, 32 * mcol),
                    )
                xct_sb = xctp.tile([128, TOK], F32, tag="xct")
                nc.vector.tensor_copy(out=xct_sb[:, :], in_=ps_xct[:, :])
                for t in range(NT):
                    ps_tr = psp.tile([128, 128], F32, tag="ps")
                    nc.tensor.transpose(
                        ps_tr[:, :], xct_sb[:, 128 * t:128 * (t + 1)],
                        identf[:, :],
                    )
                    nc.vector.scalar_tensor_tensor(
                        out=s_tiles[t][:, 128 * j:128 * (j + 1)],
                        in0=ps_tr[:, :],
                        scalar=-2.0,
                        in1=c2_sb[:, 128 * j:128 * (j + 1)],
                        op0=mybir.AluOpType.mult,
                        op1=mybir.AluOpType.add,
                    )

            kt_sb = constp.tile([NC, TOK], BF16, tag="kt_sb")
            for t in range(NT):
                score = s_tiles[t]
                score3 = score[:, :].rearrange("p (c k) -> p c k", k=K)
                m = workp.tile([128, NC], F32, tag="m")
                nc.vector.tensor_reduce(
                    out=m[:, :], in_=score3, axis=mybir.AxisListType.X,
                    op=mybir.AluOpType.min,
                )
                eq = workp.tile([128, CK], F32, tag="eq")
                nc.vector.tensor_tensor(
                    out=eq[:, :].rearrange("p (c k) -> p c k", k=K),
                    in0=score3,
                    in1=m[:, :].unsqueeze(2).broadcast_to([128, NC, K]),
                    op=mybir.AluOpType.is_equal,
                )
                cand = workp.tile([128, CK], F32, tag="cand")
                nc.vector.scalar_tensor_tensor(
                    out=cand[:, :], in0=eq[:, :], scalar=-BIG,
                    in1=iota_sb[:, :],
                    op0=mybir.AluOpType.mult, op1=mybir.AluOpType.add,
                )
                kmin = workp.tile([128, NC], F32, tag="kmin")
                nc.vector.tensor_reduce(
                    out=kmin[:, :],
                    in_=cand[:, :].rearrange("p (c k) -> p c k", k=K),
                    axis=mybir.AxisListType.X, op=mybir.AluOpType.min,
                )
                kminb = workp.tile([128, NC], BF16, tag="kminb")
                nc.vector.tensor_scalar_add(kminb[:, :], kmin[:, :], BIG)
                # transpose kmin [n, c] -> [c, n] (values 0..15, exact bf16)
                ps_kt = psp.tile([NC, 128], BF16, tag="ps")
                nc.tensor.transpose(ps_kt[:, :], kminb[:, :], identb[:, :])
                nc.vector.tensor_copy(
                    out=kt_sb[:, 128 * t:128 * (t + 1)], in_=ps_kt[:, :])

            # ---- phase 2: AllGather the indices (64KB/core) ----------------
            nc.gpsimd.dma_start(out=kt_bounce[:, :], in_=kt_sb[:, :])
            nc.gpsimd.collective_compute(
                "AllGather",
                mybir.AluOpType.bypass,
                replica_groups=[list(range(N_CORES))],
                ins=[kt_bounce.opt()],
                outs=[kt_all.opt()],
            )

            # ---- phase 3: lut = centroids @ weight ------------------------
            # 3-pass bf16 hi/lo (cbdh*wh + cbdh*wl + cbdl*wh): ~2^-18 per
            # product, bf16 matmul speed, col-packs 4 pairs per PSUM bank.
            lut_sb = lutp.tile([128, NCHUNK * OSH], F32, tag="lut")
            for j in range(NCHUNK):
                ps_lut = psp.tile([128, OSH], F32, tag="ps")
                for mcol in range(4):
                    p = 4 * j + mcol
                    wh_t = wp.tile([128, OSH], BF16, tag="wht")
                    nc.scalar.dma_start(
                        out=wh_t[:, :],
                        in_=wsh_h.ap()[128 * p:128 * (p + 1), :])
                    wl_t = wp.tile([128, OSH], BF16, tag="wlt")
                    nc.scalar.dma_start(
                        out=wl_t[:, :],
                        in_=wsh_l.ap()[128 * p:128 * (p + 1), :])
                    passes = [(cbdh_sb, wh_t), (cbdh_sb, wl_t),
                              (cbdl_sb, wh_t)]
                    for i, (cb, wt) in enumerate(passes):
                        nc.tensor.matmul(
                            ps_lut[32 * mcol:32 * (mcol + 1), :],
                            lhsT=cb[:, 32 * p:32 * (p + 1)],
                            rhs=wt[:, :],
                            start=(i == 0), stop=(i == 2),
                            tile_position=(0, 32 * mcol),
                        )
                nc.vector.tensor_copy(
                    out=lut_sb[:, OSH * j:OSH * (j + 1)], in_=ps_lut[:, :])

            # ---- phase 4: global absmax -> AllReduce(max) -> scale ---------
            mx8 = constp.tile([128, NCHUNK], F32, tag="mx8")
            for j in range(NCHUNK):
                nc.vector.tensor_reduce(
                    out=mx8[:, j:j + 1],
                    in_=lut_sb[:, OSH * j:OSH * (j + 1)],
                    axis=mybir.AxisListType.X, op=mybir.AluOpType.max,
                    apply_absolute_value=True,
                )
            mxl = constp.tile([128, 1], F32, tag="mxl")
            nc.vector.tensor_reduce(
                out=mxl[:, :], in_=mx8[:, :], axis=mybir.AxisListType.X,
                op=mybir.AluOpType.max,
            )
            mxp = constp.tile([128, 1], F32, tag="mxp")
            nc.gpsimd.partition_all_reduce(
                mxp[:, :], mxl[:, :], channels=128,
                reduce_op=bass_isa.ReduceOp.max,
            )
            zrow = constp.tile([1, 16], F32, tag="zrow")
            nc.vector.memset(zrow[:, :], 0.0)
            nc.vector.tensor_copy(out=zrow[0:1, 0:1], in_=mxp[0:1, 0:1])
            nc.sync.dma_start(out=mx_in[:, :], in_=zrow[:, :])
            nc.gpsimd.collective_compute(
                "AllReduce",
                mybir.AluOpType.max,
                replica_groups=[list(range(N_CORES))],
                ins=[mx_in.opt()],
                outs=[mx_out.opt()],
            )
            gmax_row = constp.tile([1, 16], F32, tag="gmax_row")
            nc.sync.dma_start(out=gmax_row[:, :], in_=mx_out[:, :])
            gmax = constp.tile([128, 1], F32, tag="gmax")
            nc.gpsimd.partition_broadcast(gmax[:, :], gmax_row[0:1, 0:1])
            # s = gmax/127 and 1/s via reciprocal (DVE has no divide); the
            # <=2ulp drift is far inside the error budget.
            s_col = constp.tile([128, 1], F32, tag="s_col")
            nc.vector.tensor_scalar(
                out=s_col[:, :], in0=gmax[:, :],
                scalar1=float(np.float32(1.0) / np.float32(127.0)),
                scalar2=None, op0=mybir.AluOpType.mult,
            )
            rgmax = constp.tile([128, 1], F32, tag="rgmax")
            nc.vector.reciprocal(rgmax[:, :], gmax[:, :])
            inv_s = constp.tile([128, 1], F32, tag="inv_s")
            nc.vector.tensor_scalar(
                out=inv_s[:, :], in0=rgmax[:, :], scalar1=127.0, scalar2=None,
                op0=mybir.AluOpType.mult,
            )

            # ---- phase 5: quantize-dequantize lut -> bf16 ------------------
            lutdq = lutp.tile([128, NCHUNK * OSH], BF16, tag="lutdq")
            for j in range(NCHUNK):
                qm = wp.tile([128, OSH], F32, tag="qm")
                nc.vector.tensor_scalar(
                    out=qm[:, :], in0=lut_sb[:, OSH * j:OSH * (j + 1)],
                    scalar1=inv_s[:, 0:1], scalar2=MAGIC,
                    op0=mybir.AluOpType.mult, op1=mybir.AluOpType.add,
                )
                nc.vector.tensor_scalar(
                    out=lutdq[:, OSH * j:OSH * (j + 1)], in0=qm[:, :],
                    scalar1=MAGIC, scalar2=s_col[:, 0:1],
                    op0=mybir.AluOpType.subtract, op1=mybir.AluOpType.mult,
                )

            # ---- phase 6: expand one-hots + gather matmul ------------------
            for r in range(N_CORES):
                ktr = stgp.tile([NC, TOK], BF16, tag="ktr", bufs=4)
                nc.gpsimd.dma_start(
                    out=ktr[:, :],
                    in_=kt_all[NC * r:NC * (r + 1), :])
                stg_tiles = []
                for j in range(NCHUNK):
                    # replicate kmin rows onto the 16 k-partitions of chunk j
                    ps_rep = psp.tile([128, TOK], F32, tag="ps")
                    nc.tensor.matmul(
                        ps_rep[:, :],
                        lhsT=emat_sb[:, 128 * j:128 * (j + 1)],
                        rhs=ktr[:, :],
                        start=True, stop=True,
                    )
                    stg_t = stgp.tile([128, TOK], BF16, tag="stg")
                    nc.vector.tensor_scalar(
                        out=stg_t[:, :], in0=ps_rep[:, :],
                        scalar1=kcol_sb[:, 0:1], scalar2=None,
                        op0=mybir.AluOpType.is_equal,
                    )
                    stg_tiles.append(stg_t)
                for t in range(NT):
                    ps_o = psp.tile([128, OSH], F32, tag="ps")
                    for j in range(NCHUNK):
                        nc.tensor.matmul(
                            ps_o[:, :],
                            lhsT=stg_tiles[j][:, 128 * t:128 * (t + 1)],
                            rhs=lutdq[:, OSH * j:OSH * (j + 1)],
                            start=(j == 0), stop=(j == NCHUNK - 1),
                        )
                    o_sb = outp.tile([128, OSH], BF16, tag="outsb")
                    nc.vector.tensor_tensor(
                        out=o_sb[:, :], in0=ps_o[:, :], in1=bias_sb[:, :],
                        op=mybir.AluOpType.add,
                    )
                    row0 = TOK * r + 128 * t
                    nc.scalar.dma_start(
                        out=out.ap()[row0:row0 + 128, :], in_=o_sb[:, :])

    nc.compile()
    return nc


def _prep_inputs(x, centroids, weight, bias):
    import ml_dtypes

    x = np.ascontiguousarray(np.asarray(x, dtype=np.float32)).reshape(BN, IN_F)
    cent = np.asarray(centroids, dtype=np.float32)
    w = np.asarray(weight, dtype=np.float32)
    bias = np.asarray(bias, dtype=np.float32)

    c2 = (cent ** 2).sum(axis=-1).reshape(CK)  # [1024] flat (c,k)
    c2r = np.ascontiguousarray(np.broadcast_to(c2, (128, CK)))
    iota = np.tile(np.arange(K, dtype=np.float32), NC)
    iotar = np.ascontiguousarray(np.broadcast_to(iota, (128, CK)))
    cbd = np.zeros((128, CK), np.float32)
    for p in range(NPAIR):
        cbd[0:SUBV, 32 * p:32 * p + K] = cent[2 * p].T
        cbd[SUBV:128, 32 * p + K:32 * p + 2 * K] = cent[2 * p + 1].T
    cbd_h = cbd.astype(ml_dtypes.bfloat16)
    cbd_l = (cbd - cbd_h.astype(np.float32)).astype(ml_dtypes.bfloat16)
    # E[c, ck] = 1 where ck // 16 == c  (replication matrix)
    emat = (np.arange(CK)[None, :] // K == np.arange(NC)[:, None]).astype(
        ml_dtypes.bfloat16)
    kcol = np.ascontiguousarray(
        (np.arange(128, dtype=np.float32) % K).reshape(128, 1))

    in_maps = []
    for r in range(N_CORES):
        xT_r = np.ascontiguousarray(x[TOK * r:TOK * (r + 1)].T)
        w_r = np.ascontiguousarray(w[:, :, OSH * r:OSH * (r + 1)]).reshape(
            IN_F, OSH)
        w_h = w_r.astype(ml_dtypes.bfloat16)
        w_l = (w_r - w_h.astype(np.float32)).astype(ml_dtypes.bfloat16)
        bias_r = np.ascontiguousarray(
            np.broadcast_to(bias[OSH * r:OSH * (r + 1)], (128, OSH)))
        in_maps.append({
            "xT": xT_r, "cbd": cbd, "c2r": c2r, "iotar": iotar,
            "wsh_h": w_h, "wsh_l": w_l, "cbd_h": cbd_h, "cbd_l": cbd_l,
            "biasr": bias_r, "emat": emat, "kcol": kcol,
        })
    return in_maps


def kernel(x, centroids, weight, inverse_temperature_logit, bias,
           **_unused) -> np.ndarray:
    if "nc" not in _CACHE:
        _CACHE["nc"] = _build()
    nc = _CACHE["nc"]
    in_maps = _prep_inputs(x, centroids, weight, bias)
    res = run_bass_kernel_spmd(nc, in_maps, core_ids=list(range(N_CORES)))
    out = np.concatenate(
        [res.results[r]["out"].astype(np.float32) for r in range(N_CORES)],
        axis=1)
    return out.reshape(2, BN // 2, OUT_F)


# revision 10
# speedup vs baseline: 1.0841x; 1.0841x over previous
"""AMMLinear (VQ codebook) forward on 8 TRN2 NeuronCores.

The straight-through estimator makes the forward VALUE exactly
    out[n, o] = sum_c lut_dq[c, argmin_k dist(x_cn, cent_ck), o] + bias[o]
with lut = centroids @ weight (per codebook) and lut_dq a global-scale int8
quantize-dequantize of lut.  The softmax/attention path only shapes gradients.

Sharding: tokens (BN=4096 -> 512/core) for the score/argmin phase, output
features (4096 -> 512/core) for the lut/gather phase.  One AllGather moves the
bf16 argmin indices (64KB/core), one AllReduce-max the quantization scale.
Every core then expands all 4096 tokens' one-hot codes locally (replication
matmul + is_equal) and computes its o-shard of the gather matmul.
Output is assembled host-side by concatenating the per-core o-shards.
"""

import numpy as np

import concourse.bass as bass
import concourse.mybir as mybir
import concourse.tile as tile
import concourse.bass_isa as bass_isa
from concourse import bacc
from concourse.bass_utils import run_bass_kernel_spmd
from concourse.masks import make_identity

F32 = mybir.dt.float32
F32R = mybir.dt.float32r
BF16 = mybir.dt.bfloat16

N_CORES = 8
NC, K, IN_F, OUT_F = 64, 16, 4096, 4096
SUBV = IN_F // NC          # 64
BN = 4096                  # 2*2048 tokens
TOK = BN // N_CORES        # 512 tokens per core
NT = TOK // 128            # 4 token tiles per core
NPAIR = NC // 2            # 32 codebook pairs
CK = NC * K                # 1024 (codebook,centroid) flat index
NCHUNK = CK // 128         # 8 contraction chunks
OSH = OUT_F // N_CORES     # 512 out features per core
MAGIC = 12582912.0         # 1.5 * 2**23: fp32 round-to-nearest-even trick
BIG = 1024.0

_CACHE = {}


def _build():
    nc = bacc.Bacc("TRN2", target_bir_lowering=False, debug=False,
                   num_devices=N_CORES)

    xT = nc.declare_dram_parameter("xT", [IN_F, TOK], F32, isOutput=False)
    cbd = nc.declare_dram_parameter("cbd", [128, CK], F32, isOutput=False)
    c2r = nc.declare_dram_parameter("c2r", [128, CK], F32, isOutput=False)
    iotar = nc.declare_dram_parameter("iotar", [128, CK], F32, isOutput=False)
    wsh_h = nc.declare_dram_parameter("wsh_h", [IN_F, OSH], BF16, isOutput=False)
    wsh_l = nc.declare_dram_parameter("wsh_l", [IN_F, OSH], BF16, isOutput=False)
    cbd_h = nc.declare_dram_parameter("cbd_h", [128, CK], BF16, isOutput=False)
    cbd_l = nc.declare_dram_parameter("cbd_l", [128, CK], BF16, isOutput=False)
    biasr = nc.declare_dram_parameter("biasr", [128, OSH], F32, isOutput=False)
    emat = nc.declare_dram_parameter("emat", [NC, CK], BF16, isOutput=False)
    kcol = nc.declare_dram_parameter("kcol", [128, 1], F32, isOutput=False)
    out = nc.declare_dram_parameter("out", [BN, OSH], BF16, isOutput=True)

    with tile.TileContext(nc) as tc:
        with (
            tc.tile_pool(name="consts", bufs=1) as constp,
            tc.tile_pool(name="xt", bufs=8) as xtp,
            tc.tile_pool(name="wt", bufs=6) as wp,
            tc.tile_pool(name="xct", bufs=3) as xctp,
            tc.tile_pool(name="work", bufs=2) as workp,
            tc.tile_pool(name="stg", bufs=16) as stgp,
            tc.tile_pool(name="lut", bufs=1) as lutp,
            tc.tile_pool(name="outs", bufs=4) as outp,
            tc.tile_pool(name="ps", bufs=8, space="PSUM") as psp,
            tc.tile_pool(name="dram", bufs=1, space="DRAM") as dramp,
        ):
            # ---- constants -------------------------------------------------
            cbd_sb = constp.tile([128, CK], F32, tag="cbd_sb")
            nc.sync.dma_start(out=cbd_sb[:, :], in_=cbd.ap()[:, :])
            c2_sb = constp.tile([128, CK], F32, tag="c2_sb")
            nc.sync.dma_start(out=c2_sb[:, :], in_=c2r.ap()[:, :])
            iota_sb = constp.tile([128, CK], F32, tag="iota_sb")
            nc.sync.dma_start(out=iota_sb[:, :], in_=iotar.ap()[:, :])
            bias_sb = constp.tile([128, OSH], F32, tag="bias_sb")
            nc.sync.dma_start(out=bias_sb[:, :], in_=biasr.ap()[:, :])
            emat_sb = constp.tile([NC, CK], BF16, tag="emat_sb")
            nc.sync.dma_start(out=emat_sb[:, :], in_=emat.ap()[:, :])
            kcol_sb = constp.tile([128, 1], F32, tag="kcol_sb")
            nc.sync.dma_start(out=kcol_sb[:, :], in_=kcol.ap()[:, :])
            identb = constp.tile([128, 128], BF16, tag="identb")
            make_identity(nc, identb[:, :])
            identf = constp.tile([128, 128], F32, tag="identf")
            make_identity(nc, identf[:, :])
            # bf16 hi/lo centroid blockdiag for the 3-pass lut matmul
            cbdh_sb = constp.tile([128, CK], BF16, tag="cbdh_sb")
            nc.sync.dma_start(out=cbdh_sb[:, :], in_=cbd_h.ap()[:, :])
            cbdl_sb = constp.tile([128, CK], BF16, tag="cbdl_sb")
            nc.sync.dma_start(out=cbdl_sb[:, :], in_=cbd_l.ap()[:, :])

            # DRAM scratch for collectives
            kt_bounce = dramp.tile([NC, TOK], BF16, tag="kt_bounce")
            kt_all = dramp.tile([N_CORES * NC, TOK], BF16, tag="kt_all")
            mx_in = dramp.tile([1, 16], F32, tag="mx_in")
            mx_out = dramp.tile([1, 16], F32, tag="mx_out")

            # ---- phase 1: scores (transposed), argmin, kmin^T --------------
            # xc^T[ck, n] via cent-stationary fp32 matmuls (4 pairs col-packed
            # per 128-row chunk), PE-transposed back to score[n, ck].
            s_tiles = [workp.tile([128, CK], F32, tag=f"score{t}",
                                  name=f"score{t}") for t in range(NT)]
            for j in range(NCHUNK):
                ps_xct = psp.tile([128, TOK], F32, tag="ps")
                for mcol in range(4):
                    p = 4 * j + mcol
                    xt_t = xtp.tile([128, TOK], F32, tag="xt")
                    nc.sync.dma_start(out=xt_t[:, :],
                                      in_=xT.ap()[128 * p:128 * (p + 1), :])
                    nc.tensor.matmul(
                        ps_xct[32 * mcol:32 * (mcol + 1), :],
                        lhsT=cbd_sb[:, 32 * p:32 * (p + 1)],
                        rhs=xt_t[:, :],
                        start=True, stop=True,
                        tile_position=(0, 32 * mcol),
                    )
                xct_sb = xctp.tile([128, TOK], F32, tag="xct")
                nc.vector.tensor_copy(out=xct_sb[:, :], in_=ps_xct[:, :])
                for t in range(NT):
                    ps_tr = psp.tile([128, 128], F32, tag="ps")
                    nc.tensor.transpose(
                        ps_tr[:, :], xct_sb[:, 128 * t:128 * (t + 1)],
                        identf[:, :],
                    )
                    nc.vector.scalar_tensor_tensor(
                        out=s_tiles[t][:, 128 * j:128 * (j + 1)],
                        in0=ps_tr[:, :],
                        scalar=-2.0,
                        in1=c2_sb[:, 128 * j:128 * (j + 1)],
                        op0=mybir.AluOpType.mult,
                        op1=mybir.AluOpType.add,
                    )

            kt_sb = constp.tile([NC, TOK], BF16, tag="kt_sb")
            for t in range(NT):
                score = s_tiles[t]
                score3 = score[:, :].rearrange("p (c k) -> p c k", k=K)
                m = workp.tile([128, NC], F32, tag="m")
                nc.vector.tensor_reduce(
                    out=m[:, :], in_=score3, axis=mybir.AxisListType.X,
                    op=mybir.AluOpType.min,
                )
                eq = workp.tile([128, CK], F32, tag="eq")
                nc.vector.tensor_tensor(
                    out=eq[:, :].rearrange("p (c k) -> p c k", k=K),
                    in0=score3,
                    in1=m[:, :].unsqueeze(2).broadcast_to([128, NC, K]),
                    op=mybir.AluOpType.is_equal,
                )
                cand = workp.tile([128, CK], F32, tag="cand")
                nc.vector.scalar_tensor_tensor(
                    out=cand[:, :], in0=eq[:, :], scalar=-BIG,
                    in1=iota_sb[:, :],
                    op0=mybir.AluOpType.mult, op1=mybir.AluOpType.add,
                )
                kmin = workp.tile([128, NC], F32, tag="kmin")
                nc.vector.tensor_reduce(
                    out=kmin[:, :],
                    in_=cand[:, :].rearrange("p (c k) -> p c k", k=K),
                    axis=mybir.AxisListType.X, op=mybir.AluOpType.min,
                )
                kminb = workp.tile([128, NC], BF16, tag="kminb")
                nc.vector.tensor_scalar_add(kminb[:, :], kmin[:, :], BIG)
                # transpose kmin [n, c] -> [c, n] (values 0..15, exact bf16)
                ps_kt = psp.tile([NC, 128], BF16, tag="ps")
                nc.tensor.transpose(ps_kt[:, :], kminb[:, :], identb[:, :])
                nc.vector.tensor_copy(
                    out=kt_sb[:, 128 * t:128 * (t + 1)], in_=ps_kt[:, :])

            # ---- phase 2: AllGather the indices (64KB/core) ----------------
            nc.sync.dma_start(out=kt_bounce[:, :], in_=kt_sb[:, :])
            nc.gpsimd.collective_compute(
                "AllGather",
                mybir.AluOpType.bypass,
                replica_groups=[list(range(N_CORES))],
                ins=[kt_bounce.opt()],
                outs=[kt_all.opt()],
            )

            # ---- phase 3: lut = centroids @ weight ------------------------
            # 3-pass bf16 hi/lo (cbdh*wh + cbdh*wl + cbdl*wh): ~2^-18 per
            # product, bf16 matmul speed, col-packs 4 pairs per PSUM bank.
            lut_sb = lutp.tile([128, NCHUNK * OSH], F32, tag="lut")
            for j in range(NCHUNK):
                ps_lut = psp.tile([128, OSH], F32, tag="ps")
                for mcol in range(4):
                    p = 4 * j + mcol
                    wh_t = wp.tile([128, OSH], BF16, tag="wht")
                    nc.sync.dma_start(
                        out=wh_t[:, :],
                        in_=wsh_h.ap()[128 * p:128 * (p + 1), :])
                    wl_t = wp.tile([128, OSH], BF16, tag="wlt")
                    nc.sync.dma_start(
                        out=wl_t[:, :],
                        in_=wsh_l.ap()[128 * p:128 * (p + 1), :])
                    passes = [(cbdh_sb, wh_t), (cbdh_sb, wl_t),
                              (cbdl_sb, wh_t)]
                    for i, (cb, wt) in enumerate(passes):
                        nc.tensor.matmul(
                            ps_lut[32 * mcol:32 * (mcol + 1), :],
                            lhsT=cb[:, 32 * p:32 * (p + 1)],
                            rhs=wt[:, :],
                            start=(i == 0), stop=(i == 2),
                            tile_position=(0, 32 * mcol),
                        )
                nc.vector.tensor_copy(
                    out=lut_sb[:, OSH * j:OSH * (j + 1)], in_=ps_lut[:, :])

            # ---- phase 4: global absmax -> AllReduce(max) -> scale ---------
            mx8 = constp.tile([128, NCHUNK], F32, tag="mx8")
            for j in range(NCHUNK):
                nc.vector.tensor_reduce(
                    out=mx8[:, j:j + 1],
                    in_=lut_sb[:, OSH * j:OSH * (j + 1)],
                    axis=mybir.AxisListType.X, op=mybir.AluOpType.max,
                    apply_absolute_value=True,
                )
            mxl = constp.tile([128, 1], F32, tag="mxl")
            nc.vector.tensor_reduce(
                out=mxl[:, :], in_=mx8[:, :], axis=mybir.AxisListType.X,
                op=mybir.AluOpType.max,
            )
            mxp = constp.tile([128, 1], F32, tag="mxp")
            nc.gpsimd.partition_all_reduce(
                mxp[:, :], mxl[:, :], channels=128,
                reduce_op=bass_isa.ReduceOp.max,
            )
            zrow = constp.tile([1, 16], F32, tag="zrow")
            nc.vector.memset(zrow[:, :], 0.0)
            nc.vector.tensor_copy(out=zrow[0:1, 0:1], in_=mxp[0:1, 0:1])
            nc.sync.dma_start(out=mx_in[:, :], in_=zrow[:, :])
            nc.gpsimd.collective_compute(
                "AllReduce",
                mybir.AluOpType.max,
                replica_groups=[list(range(N_CORES))],
                ins=[mx_in.opt()],
                outs=[mx_out.opt()],
            )
            gmax_row = constp.tile([1, 16], F32, tag="gmax_row")
            nc.sync.dma_start(out=gmax_row[:, :], in_=mx_out[:, :])
            gmax = constp.tile([128, 1], F32, tag="gmax")
            nc.gpsimd.partition_broadcast(gmax[:, :], gmax_row[0:1, 0:1])
            # s = gmax/127 and 1/s via reciprocal (DVE has no divide); the
            # <=2ulp drift is far inside the error budget.
            s_col = constp.tile([128, 1], F32, tag="s_col")
            nc.vector.tensor_scalar(
                out=s_col[:, :], in0=gmax[:, :],
                scalar1=float(np.float32(1.0) / np.float32(127.0)),
                scalar2=None, op0=mybir.AluOpType.mult,
            )
            rgmax = constp.tile([128, 1], F32, tag="rgmax")
            nc.vector.reciprocal(rgmax[:, :], gmax[:, :])
            inv_s = constp.tile([128, 1], F32, tag="inv_s")
            nc.vector.tensor_scalar(
                out=inv_s[:, :], in0=rgmax[:, :], scalar1=127.0, scalar2=None,
                op0=mybir.AluOpType.mult,
            )

            # ---- phase 5: quantize-dequantize lut -> bf16 ------------------
            lutdq = lutp.tile([128, NCHUNK * OSH], BF16, tag="lutdq")
            for j in range(NCHUNK):
                qm = wp.tile([128, OSH], F32, tag="qm")
                nc.vector.tensor_scalar(
                    out=qm[:, :], in0=lut_sb[:, OSH * j:OSH * (j + 1)],
                    scalar1=inv_s[:, 0:1], scalar2=MAGIC,
                    op0=mybir.AluOpType.mult, op1=mybir.AluOpType.add,
                )
                nc.vector.tensor_scalar(
                    out=lutdq[:, OSH * j:OSH * (j + 1)], in0=qm[:, :],
                    scalar1=MAGIC, scalar2=s_col[:, 0:1],
                    op0=mybir.AluOpType.subtract, op1=mybir.AluOpType.mult,
                )

            # ---- phase 6: expand one-hots + gather matmul ------------------
            for r in range(N_CORES):
                ktr = stgp.tile([NC, TOK], BF16, tag="ktr", bufs=4)
                nc.sync.dma_start(
                    out=ktr[:, :],
                    in_=kt_all[NC * r:NC * (r + 1), :])
                stg_tiles = []
                for j in range(NCHUNK):
                    # replicate kmin rows onto the 16 k-partitions of chunk j
                    ps_rep = psp.tile([128, TOK], F32, tag="ps")
                    nc.tensor.matmul(
                        ps_rep[:, :],
                        lhsT=emat_sb[:, 128 * j:128 * (j + 1)],
                        rhs=ktr[:, :],
                        start=True, stop=True,
                    )
                    stg_t = stgp.tile([128, TOK], BF16, tag="stg")
                    nc.vector.tensor_scalar(
                        out=stg_t[:, :], in0=ps_rep[:, :],
                        scalar1=kcol_sb[:, 0:1], scalar2=None,
                        op0=mybir.AluOpType.is_equal,
                    )
                    stg_tiles.append(stg_t)
                for t in range(NT):
                    ps_o = psp.tile([128, OSH], F32, tag="ps")
                    for j in range(NCHUNK):
                        nc.tensor.matmul(
                            ps_o[:, :],
                            lhsT=stg_tiles[j][:, 128 * t:128 * (t + 1)],
                            rhs=lutdq[:, OSH * j:OSH * (j + 1)],
                            start=(j == 0), stop=(j == NCHUNK - 1),
                        )
                    o_sb = outp.tile([128, OSH], BF16, tag="outsb")
                    nc.vector.tensor_tensor(
                        out=o_sb[:, :], in0=ps_o[:, :], in1=bias_sb[:, :],
                        op=mybir.AluOpType.add,
                    )
                    row0 = TOK * r + 128 * t
                    nc.sync.dma_start(
                        out=out.ap()[row0:row0 + 128, :], in_=o_sb[:, :])

    nc.compile()
    return nc


def _prep_inputs(x, centroids, weight, bias):
    import ml_dtypes

    x = np.ascontiguousarray(np.asarray(x, dtype=np.float32)).reshape(BN, IN_F)
    cent = np.asarray(centroids, dtype=np.float32)
    w = np.asarray(weight, dtype=np.float32)
    bias = np.asarray(bias, dtype=np.float32)

    c2 = (cent ** 2).sum(axis=-1).reshape(CK)  # [1024] flat (c,k)
    c2r = np.ascontiguousarray(np.broadcast_to(c2, (128, CK)))
    iota = np.tile(np.arange(K, dtype=np.float32), NC)
    iotar = np.ascontiguousarray(np.broadcast_to(iota, (128, CK)))
    cbd = np.zeros((128, CK), np.float32)
    for p in range(NPAIR):
        cbd[0:SUBV, 32 * p:32 * p + K] = cent[2 * p].T
        cbd[SUBV:128, 32 * p + K:32 * p + 2 * K] = cent[2 * p + 1].T
    cbd_h = cbd.astype(ml_dtypes.bfloat16)
    cbd_l = (cbd - cbd_h.astype(np.float32)).astype(ml_dtypes.bfloat16)
    # E[c, ck] = 1 where ck // 16 == c  (replication matrix)
    emat = (np.arange(CK)[None, :] // K == np.arange(NC)[:, None]).astype(
        ml_dtypes.bfloat16)
    kcol = np.ascontiguousarray(
        (np.arange(128, dtype=np.float32) % K).reshape(128, 1))

    in_maps = []
    for r in range(N_CORES):
        xT_r = np.ascontiguousarray(x[TOK * r:TOK * (r + 1)].T)
        w_r = np.ascontiguousarray(w[:, :, OSH * r:OSH * (r + 1)]).reshape(
            IN_F, OSH)
        w_h = w_r.astype(ml_dtypes.bfloat16)
        w_l = (w_r - w_h.astype(np.float32)).astype(ml_dtypes.bfloat16)
        bias_r = np.ascontiguousarray(
            np.broadcast_to(bias[OSH * r:OSH * (r + 1)], (128, OSH)))
        in_maps.append({
            "xT": xT_r, "cbd": cbd, "c2r": c2r, "iotar": iotar,
            "wsh_h": w_h, "wsh_l": w_l, "cbd_h": cbd_h, "cbd_l": cbd_l,
            "biasr": bias_r, "emat": emat, "kcol": kcol,
        })
    return in_maps


def kernel(x, centroids, weight, inverse_temperature_logit, bias,
           **_unused) -> np.ndarray:
    if "nc" not in _CACHE:
        _CACHE["nc"] = _build()
    nc = _CACHE["nc"]
    in_maps = _prep_inputs(x, centroids, weight, bias)
    res = run_bass_kernel_spmd(nc, in_maps, core_ids=list(range(N_CORES)))
    out = np.concatenate(
        [res.results[r]["out"].astype(np.float32) for r in range(N_CORES)],
        axis=1)
    return out.reshape(2, BN // 2, OUT_F)


# revision 11
# speedup vs baseline: 1.1188x; 1.0321x over previous
"""AMMLinear (VQ codebook) forward on 8 TRN2 NeuronCores.

The straight-through estimator makes the forward VALUE exactly
    out[n, o] = sum_c lut_dq[c, argmin_k dist(x_cn, cent_ck), o] + bias[o]
with lut = centroids @ weight (per codebook) and lut_dq a global-scale int8
quantize-dequantize of lut.  The softmax/attention path only shapes gradients.

Sharding: tokens (BN=4096 -> 512/core) for the score/argmin phase, output
features (4096 -> 512/core) for the lut/gather phase.  One AllGather moves the
bf16 argmin indices (64KB/core), one AllReduce-max the quantization scale.
Every core then expands all 4096 tokens' one-hot codes locally (replication
matmul + is_equal) and computes its o-shard of the gather matmul.
Output is assembled host-side by concatenating the per-core o-shards.
"""

import numpy as np

import concourse.bass as bass
import concourse.mybir as mybir
import concourse.tile as tile
import concourse.bass_isa as bass_isa
from concourse import bacc
from concourse.bass_utils import run_bass_kernel_spmd
from concourse.masks import make_identity

F32 = mybir.dt.float32
F32R = mybir.dt.float32r
BF16 = mybir.dt.bfloat16

N_CORES = 8
NC, K, IN_F, OUT_F = 64, 16, 4096, 4096
SUBV = IN_F // NC          # 64
BN = 4096                  # 2*2048 tokens
TOK = BN // N_CORES        # 512 tokens per core
NT = TOK // 128            # 4 token tiles per core
NPAIR = NC // 2            # 32 codebook pairs
CK = NC * K                # 1024 (codebook,centroid) flat index
NCHUNK = CK // 128         # 8 contraction chunks
OSH = OUT_F // N_CORES     # 512 out features per core
MAGIC = 12582912.0         # 1.5 * 2**23: fp32 round-to-nearest-even trick
BIG = 1024.0

_CACHE = {}


def _build():
    nc = bacc.Bacc("TRN2", target_bir_lowering=False, debug=False,
                   num_devices=N_CORES)

    xT = nc.declare_dram_parameter("xT", [IN_F, TOK], F32, isOutput=False)
    cbd = nc.declare_dram_parameter("cbd", [128, CK], F32, isOutput=False)
    c2r = nc.declare_dram_parameter("c2r", [128, CK], F32, isOutput=False)
    iotar = nc.declare_dram_parameter("iotar", [128, 128], F32, isOutput=False)
    wsh_h = nc.declare_dram_parameter("wsh_h", [IN_F, OSH], BF16, isOutput=False)
    wsh_l = nc.declare_dram_parameter("wsh_l", [IN_F, OSH], BF16, isOutput=False)
    cbd_h = nc.declare_dram_parameter("cbd_h", [128, CK], BF16, isOutput=False)
    cbd_l = nc.declare_dram_parameter("cbd_l", [128, CK], BF16, isOutput=False)
    biasr = nc.declare_dram_parameter("biasr", [128, OSH], F32, isOutput=False)
    emat = nc.declare_dram_parameter("emat", [NC, CK], BF16, isOutput=False)
    kcol = nc.declare_dram_parameter("kcol", [128, 1], F32, isOutput=False)
    out = nc.declare_dram_parameter("out", [BN, OSH], BF16, isOutput=True)

    with tile.TileContext(nc) as tc:
        with (
            tc.tile_pool(name="consts", bufs=1) as constp,
            tc.tile_pool(name="xt", bufs=8) as xtp,
            tc.tile_pool(name="wt", bufs=6) as wp,
            tc.tile_pool(name="xct", bufs=3) as xctp,
            tc.tile_pool(name="work", bufs=2) as workp,
            tc.tile_pool(name="stg", bufs=16) as stgp,
            tc.tile_pool(name="lut", bufs=1) as lutp,
            tc.tile_pool(name="outs", bufs=4) as outp,
            tc.tile_pool(name="ps", bufs=8, space="PSUM") as psp,
            tc.tile_pool(name="dram", bufs=1, space="DRAM") as dramp,
        ):
            # ---- constants -------------------------------------------------
            cbd_sb = constp.tile([128, CK], F32, tag="cbd_sb")
            nc.sync.dma_start(out=cbd_sb[:, :], in_=cbd.ap()[:, :])
            c2_sb = constp.tile([128, CK], F32, tag="c2_sb")
            nc.sync.dma_start(out=c2_sb[:, :], in_=c2r.ap()[:, :])
            iota_sb = constp.tile([128, 128], F32, tag="iota_sb")
            nc.sync.dma_start(out=iota_sb[:, :], in_=iotar.ap()[:, :])
            identb = constp.tile([128, 128], BF16, tag="identb")
            make_identity(nc, identb[:, :])
            identf = constp.tile([128, 128], F32, tag="identf")
            make_identity(nc, identf[:, :])

            # DRAM scratch for collectives
            kt_bounce = dramp.tile([NC, TOK], BF16, tag="kt_bounce")
            kt_all = dramp.tile([N_CORES * NC, TOK], BF16, tag="kt_all")
            mx_in = dramp.tile([1, 16], F32, tag="mx_in")
            mx_out = dramp.tile([1, 16], F32, tag="mx_out")

            # ---- phase 1: scores (transposed), incremental argmin ----------
            # xc^T[ck, n] via cent-stationary fp32 matmuls (4 pairs col-packed
            # per 128-row chunk), PE-transposed back to score[n, ck].  Each
            # 128-wide chunk holds 8 complete codebooks, so the first-index
            # argmin runs per (chunk, token-tile) right behind the transpose.
            kmin_t = [workp.tile([128, NC], F32, tag=f"kmin{t}",
                                 name=f"kmin{t}") for t in range(NT)]
            for j in range(NCHUNK):
                ps_xct = psp.tile([128, TOK], F32, tag="ps")
                for mcol in range(4):
                    p = 4 * j + mcol
                    xt_t = xtp.tile([128, TOK], F32, tag="xt")
                    nc.sync.dma_start(out=xt_t[:, :],
                                      in_=xT.ap()[128 * p:128 * (p + 1), :])
                    nc.tensor.matmul(
                        ps_xct[32 * mcol:32 * (mcol + 1), :],
                        lhsT=cbd_sb[:, 32 * p:32 * (p + 1)],
                        rhs=xt_t[:, :],
                        start=True, stop=True,
                        tile_position=(0, 32 * mcol),
                    )
                xct_sb = xctp.tile([128, TOK], F32, tag="xct")
                nc.vector.tensor_copy(out=xct_sb[:, :], in_=ps_xct[:, :])
                for t in range(NT):
                    ps_tr = psp.tile([128, 128], F32, tag="ps")
                    nc.tensor.transpose(
                        ps_tr[:, :], xct_sb[:, 128 * t:128 * (t + 1)],
                        identf[:, :],
                    )
                    ssl = workp.tile([128, 128], F32, tag="ssl")
                    nc.vector.scalar_tensor_tensor(
                        out=ssl[:, :],
                        in0=ps_tr[:, :],
                        scalar=-2.0,
                        in1=c2_sb[:, 128 * j:128 * (j + 1)],
                        op0=mybir.AluOpType.mult,
                        op1=mybir.AluOpType.add,
                    )
                    ssl3 = ssl[:, :].rearrange("p (c k) -> p c k", k=K)
                    m8 = workp.tile([128, 8], F32, tag="m8")
                    nc.vector.tensor_reduce(
                        out=m8[:, :], in_=ssl3, axis=mybir.AxisListType.X,
                        op=mybir.AluOpType.min,
                    )
                    eq = workp.tile([128, 128], F32, tag="eq")
                    nc.vector.tensor_tensor(
                        out=eq[:, :].rearrange("p (c k) -> p c k", k=K),
                        in0=ssl3,
                        in1=m8[:, :].unsqueeze(2).broadcast_to([128, 8, K]),
                        op=mybir.AluOpType.is_equal,
                    )
                    cand = workp.tile([128, 128], F32, tag="cand")
                    nc.vector.scalar_tensor_tensor(
                        out=cand[:, :], in0=eq[:, :], scalar=-BIG,
                        in1=iota_sb[:, :],
                        op0=mybir.AluOpType.mult, op1=mybir.AluOpType.add,
                    )
                    nc.vector.tensor_reduce(
                        out=kmin_t[t][:, 8 * j:8 * (j + 1)],
                        in_=cand[:, :].rearrange("p (c k) -> p c k", k=K),
                        axis=mybir.AxisListType.X, op=mybir.AluOpType.min,
                    )

            kt_sb = constp.tile([NC, TOK], BF16, tag="kt_sb")
            for t in range(NT):
                kminb = workp.tile([128, NC], BF16, tag="kminb")
                nc.vector.tensor_scalar_add(kminb[:, :], kmin_t[t][:, :], BIG)
                # transpose kmin [n, c] -> [c, n] (values 0..15, exact bf16)
                ps_kt = psp.tile([NC, 128], BF16, tag="ps")
                nc.tensor.transpose(ps_kt[:, :], kminb[:, :], identb[:, :])
                nc.vector.tensor_copy(
                    out=kt_sb[:, 128 * t:128 * (t + 1)], in_=ps_kt[:, :])

            # ---- phase 2: AllGather the indices (64KB/core) ----------------
            nc.sync.dma_start(out=kt_bounce[:, :], in_=kt_sb[:, :])
            nc.gpsimd.collective_compute(
                "AllGather",
                mybir.AluOpType.bypass,
                replica_groups=[list(range(N_CORES))],
                ins=[kt_bounce.opt()],
                outs=[kt_all.opt()],
            )

            # bf16 hi/lo centroid blockdiag for the 3-pass lut matmul
            cbdh_sb = constp.tile([128, CK], BF16, tag="cbdh_sb")
            nc.sync.dma_start(out=cbdh_sb[:, :], in_=cbd_h.ap()[:, :])
            cbdl_sb = constp.tile([128, CK], BF16, tag="cbdl_sb")
            nc.sync.dma_start(out=cbdl_sb[:, :], in_=cbd_l.ap()[:, :])

            # ---- phase 3: lut = centroids @ weight ------------------------
            # 3-pass bf16 hi/lo (cbdh*wh + cbdh*wl + cbdl*wh): ~2^-18 per
            # product, bf16 matmul speed, col-packs 4 pairs per PSUM bank.
            lut_sb = lutp.tile([128, NCHUNK * OSH], F32, tag="lut")
            for j in range(NCHUNK):
                ps_lut = psp.tile([128, OSH], F32, tag="ps")
                for mcol in range(4):
                    p = 4 * j + mcol
                    wh_t = wp.tile([128, OSH], BF16, tag="wht")
                    nc.sync.dma_start(
                        out=wh_t[:, :],
                        in_=wsh_h.ap()[128 * p:128 * (p + 1), :])
                    wl_t = wp.tile([128, OSH], BF16, tag="wlt")
                    nc.sync.dma_start(
                        out=wl_t[:, :],
                        in_=wsh_l.ap()[128 * p:128 * (p + 1), :])
                    passes = [(cbdh_sb, wh_t), (cbdh_sb, wl_t),
                              (cbdl_sb, wh_t)]
                    for i, (cb, wt) in enumerate(passes):
                        nc.tensor.matmul(
                            ps_lut[32 * mcol:32 * (mcol + 1), :],
                            lhsT=cb[:, 32 * p:32 * (p + 1)],
                            rhs=wt[:, :],
                            start=(i == 0), stop=(i == 2),
                            tile_position=(0, 32 * mcol),
                        )
                nc.vector.tensor_copy(
                    out=lut_sb[:, OSH * j:OSH * (j + 1)], in_=ps_lut[:, :])

            # ---- phase 4: global absmax -> AllReduce(max) -> scale ---------
            mx8 = constp.tile([128, NCHUNK], F32, tag="mx8")
            for j in range(NCHUNK):
                nc.vector.tensor_reduce(
                    out=mx8[:, j:j + 1],
                    in_=lut_sb[:, OSH * j:OSH * (j + 1)],
                    axis=mybir.AxisListType.X, op=mybir.AluOpType.max,
                    apply_absolute_value=True,
                )
            mxl = constp.tile([128, 1], F32, tag="mxl")
            nc.vector.tensor_reduce(
                out=mxl[:, :], in_=mx8[:, :], axis=mybir.AxisListType.X,
                op=mybir.AluOpType.max,
            )
            mxp = constp.tile([128, 1], F32, tag="mxp")
            nc.gpsimd.partition_all_reduce(
                mxp[:, :], mxl[:, :], channels=128,
                reduce_op=bass_isa.ReduceOp.max,
            )
            zrow = constp.tile([1, 16], F32, tag="zrow")
            nc.vector.memset(zrow[:, :], 0.0)
            nc.vector.tensor_copy(out=zrow[0:1, 0:1], in_=mxp[0:1, 0:1])
            nc.sync.dma_start(out=mx_in[:, :], in_=zrow[:, :])
            nc.gpsimd.collective_compute(
                "AllReduce",
                mybir.AluOpType.max,
                replica_groups=[list(range(N_CORES))],
                ins=[mx_in.opt()],
                outs=[mx_out.opt()],
            )
            gmax_row = constp.tile([1, 16], F32, tag="gmax_row")
            nc.sync.dma_start(out=gmax_row[:, :], in_=mx_out[:, :])
            gmax = constp.tile([128, 1], F32, tag="gmax")
            nc.gpsimd.partition_broadcast(gmax[:, :], gmax_row[0:1, 0:1])
            # s = gmax/127 and 1/s via reciprocal (DVE has no divide); the
            # <=2ulp drift is far inside the error budget.
            s_col = constp.tile([128, 1], F32, tag="s_col")
            nc.vector.tensor_scalar(
                out=s_col[:, :], in0=gmax[:, :],
                scalar1=float(np.float32(1.0) / np.float32(127.0)),
                scalar2=None, op0=mybir.AluOpType.mult,
            )
            rgmax = constp.tile([128, 1], F32, tag="rgmax")
            nc.vector.reciprocal(rgmax[:, :], gmax[:, :])
            inv_s = constp.tile([128, 1], F32, tag="inv_s")
            nc.vector.tensor_scalar(
                out=inv_s[:, :], in0=rgmax[:, :], scalar1=127.0, scalar2=None,
                op0=mybir.AluOpType.mult,
            )

            # ---- phase 5: quantize-dequantize lut -> bf16 ------------------
            lutdq = lutp.tile([128, NCHUNK * OSH], BF16, tag="lutdq")
            for j in range(NCHUNK):
                qm = wp.tile([128, OSH], F32, tag="qm")
                nc.vector.tensor_scalar(
                    out=qm[:, :], in0=lut_sb[:, OSH * j:OSH * (j + 1)],
                    scalar1=inv_s[:, 0:1], scalar2=MAGIC,
                    op0=mybir.AluOpType.mult, op1=mybir.AluOpType.add,
                )
                nc.vector.tensor_scalar(
                    out=lutdq[:, OSH * j:OSH * (j + 1)], in0=qm[:, :],
                    scalar1=MAGIC, scalar2=s_col[:, 0:1],
                    op0=mybir.AluOpType.subtract, op1=mybir.AluOpType.mult,
                )

            bias_sb = constp.tile([128, OSH], F32, tag="bias_sb")
            nc.sync.dma_start(out=bias_sb[:, :], in_=biasr.ap()[:, :])
            emat_sb = constp.tile([NC, CK], BF16, tag="emat_sb")
            nc.sync.dma_start(out=emat_sb[:, :], in_=emat.ap()[:, :])
            kcol_sb = constp.tile([128, 1], F32, tag="kcol_sb")
            nc.sync.dma_start(out=kcol_sb[:, :], in_=kcol.ap()[:, :])

            # ---- phase 6: expand one-hots + gather matmul ------------------
            for r in range(N_CORES):
                ktr = stgp.tile([NC, TOK], BF16, tag="ktr", bufs=4)
                nc.sync.dma_start(
                    out=ktr[:, :],
                    in_=kt_all[NC * r:NC * (r + 1), :])
                stg_tiles = []
                for j in range(NCHUNK):
                    # replicate kmin rows onto the 16 k-partitions of chunk j
                    ps_rep = psp.tile([128, TOK], F32, tag="ps")
                    nc.tensor.matmul(
                        ps_rep[:, :],
                        lhsT=emat_sb[:, 128 * j:128 * (j + 1)],
                        rhs=ktr[:, :],
                        start=True, stop=True,
                    )
                    stg_t = stgp.tile([128, TOK], BF16, tag="stg")
                    nc.vector.tensor_scalar(
                        out=stg_t[:, :], in0=ps_rep[:, :],
                        scalar1=kcol_sb[:, 0:1], scalar2=None,
                        op0=mybir.AluOpType.is_equal,
                    )
                    stg_tiles.append(stg_t)
                for t in range(NT):
                    ps_o = psp.tile([128, OSH], F32, tag="ps")
                    for j in range(NCHUNK):
                        nc.tensor.matmul(
                            ps_o[:, :],
                            lhsT=stg_tiles[j][:, 128 * t:128 * (t + 1)],
                            rhs=lutdq[:, OSH * j:OSH * (j + 1)],
                            start=(j == 0), stop=(j == NCHUNK - 1),
                        )
                    o_sb = outp.tile([128, OSH], BF16, tag="outsb")
                    nc.vector.tensor_tensor(
                        out=o_sb[:, :], in0=ps_o[:, :], in1=bias_sb[:, :],
                        op=mybir.AluOpType.add,
                    )
                    row0 = TOK * r + 128 * t
                    nc.sync.dma_start(
                        out=out.ap()[row0:row0 + 128, :], in_=o_sb[:, :])

    nc.compile()
    return nc


def _prep_inputs(x, centroids, weight, bias):
    import ml_dtypes

    x = np.ascontiguousarray(np.asarray(x, dtype=np.float32)).reshape(BN, IN_F)
    cent = np.asarray(centroids, dtype=np.float32)
    w = np.asarray(weight, dtype=np.float32)
    bias = np.asarray(bias, dtype=np.float32)

    c2 = (cent ** 2).sum(axis=-1).reshape(CK)  # [1024] flat (c,k)
    c2r = np.ascontiguousarray(np.broadcast_to(c2, (128, CK)))
    iota = np.tile(np.arange(K, dtype=np.float32), 8)
    iotar = np.ascontiguousarray(np.broadcast_to(iota, (128, 128)))
    cbd = np.zeros((128, CK), np.float32)
    for p in range(NPAIR):
        cbd[0:SUBV, 32 * p:32 * p + K] = cent[2 * p].T
        cbd[SUBV:128, 32 * p + K:32 * p + 2 * K] = cent[2 * p + 1].T
    cbd_h = cbd.astype(ml_dtypes.bfloat16)
    cbd_l = (cbd - cbd_h.astype(np.float32)).astype(ml_dtypes.bfloat16)
    # E[c, ck] = 1 where ck // 16 == c  (replication matrix)
    emat = (np.arange(CK)[None, :] // K == np.arange(NC)[:, None]).astype(
        ml_dtypes.bfloat16)
    kcol = np.ascontiguousarray(
        (np.arange(128, dtype=np.float32) % K).reshape(128, 1))

    in_maps = []
    for r in range(N_CORES):
        xT_r = np.ascontiguousarray(x[TOK * r:TOK * (r + 1)].T)
        w_r = np.ascontiguousarray(w[:, :, OSH * r:OSH * (r + 1)]).reshape(
            IN_F, OSH)
        w_h = w_r.astype(ml_dtypes.bfloat16)
        w_l = (w_r - w_h.astype(np.float32)).astype(ml_dtypes.bfloat16)
        bias_r = np.ascontiguousarray(
            np.broadcast_to(bias[OSH * r:OSH * (r + 1)], (128, OSH)))
        in_maps.append({
            "xT": xT_r, "cbd": cbd, "c2r": c2r, "iotar": iotar,
            "wsh_h": w_h, "wsh_l": w_l, "cbd_h": cbd_h, "cbd_l": cbd_l,
            "biasr": bias_r, "emat": emat, "kcol": kcol,
        })
    return in_maps


def kernel(x, centroids, weight, inverse_temperature_logit, bias,
           **_unused) -> np.ndarray:
    if "nc" not in _CACHE:
        _CACHE["nc"] = _build()
    nc = _CACHE["nc"]
    in_maps = _prep_inputs(x, centroids, weight, bias)
    res = run_bass_kernel_spmd(nc, in_maps, core_ids=list(range(N_CORES)))
    out = np.concatenate(
        [res.results[r]["out"].astype(np.float32) for r in range(N_CORES)],
        axis=1)
    return out.reshape(2, BN // 2, OUT_F)


# revision 12
# speedup vs baseline: 1.1720x; 1.0475x over previous
"""AMMLinear (VQ codebook) forward on 8 TRN2 NeuronCores.

The straight-through estimator makes the forward VALUE exactly
    out[n, o] = sum_c lut_dq[c, argmin_k dist(x_cn, cent_ck), o] + bias[o]
with lut = centroids @ weight (per codebook) and lut_dq a global-scale int8
quantize-dequantize of lut.  The softmax/attention path only shapes gradients.

Sharding: tokens (BN=4096 -> 512/core) for the score/argmin phase, output
features (4096 -> 512/core) for the lut/gather phase.  One AllGather moves the
bf16 argmin indices (64KB/core), one AllReduce-max the quantization scale.
Every core then expands all 4096 tokens' one-hot codes locally (replication
matmul + is_equal) and computes its o-shard of the gather matmul.
Output is assembled host-side by concatenating the per-core o-shards.
"""

import numpy as np

import concourse.bass as bass
import concourse.mybir as mybir
import concourse.tile as tile
import concourse.bass_isa as bass_isa
from concourse import bacc
from concourse.bass_utils import run_bass_kernel_spmd
from concourse.masks import make_identity

F32 = mybir.dt.float32
F32R = mybir.dt.float32r
BF16 = mybir.dt.bfloat16

N_CORES = 8
NC, K, IN_F, OUT_F = 64, 16, 4096, 4096
SUBV = IN_F // NC          # 64
BN = 4096                  # 2*2048 tokens
TOK = BN // N_CORES        # 512 tokens per core
NT = TOK // 128            # 4 token tiles per core
NPAIR = NC // 2            # 32 codebook pairs
CK = NC * K                # 1024 (codebook,centroid) flat index
NCHUNK = CK // 128         # 8 contraction chunks
OSH = OUT_F // N_CORES     # 512 out features per core
MAGIC = 12582912.0         # 1.5 * 2**23: fp32 round-to-nearest-even trick
BIG = 1024.0

_CACHE = {}


def _build():
    nc = bacc.Bacc("TRN2", target_bir_lowering=False, debug=False,
                   num_devices=N_CORES)

    xh = nc.declare_dram_parameter("xh", [IN_F, TOK], BF16, isOutput=False)
    xl = nc.declare_dram_parameter("xl", [IN_F, TOK], BF16, isOutput=False)
    cbd = nc.declare_dram_parameter("cbd", [128, CK], F32, isOutput=False)
    c2r = nc.declare_dram_parameter("c2r", [128, CK], F32, isOutput=False)
    iotar = nc.declare_dram_parameter("iotar", [128, 128], F32, isOutput=False)
    wsh_h = nc.declare_dram_parameter("wsh_h", [IN_F, OSH], BF16, isOutput=False)
    wsh_l = nc.declare_dram_parameter("wsh_l", [IN_F, OSH], BF16, isOutput=False)
    cbd_h = nc.declare_dram_parameter("cbd_h", [128, CK], BF16, isOutput=False)
    cbd_l = nc.declare_dram_parameter("cbd_l", [128, CK], BF16, isOutput=False)
    biasr = nc.declare_dram_parameter("biasr", [128, OSH], F32, isOutput=False)
    emat = nc.declare_dram_parameter("emat", [NC, CK], BF16, isOutput=False)
    kcol = nc.declare_dram_parameter("kcol", [128, 1], F32, isOutput=False)
    out = nc.declare_dram_parameter("out", [BN, OSH], BF16, isOutput=True)

    with tile.TileContext(nc) as tc:
        with (
            tc.tile_pool(name="consts", bufs=1) as constp,
            tc.tile_pool(name="xt", bufs=8) as xtp,
            tc.tile_pool(name="wt", bufs=6) as wp,
            tc.tile_pool(name="xct", bufs=3) as xctp,
            tc.tile_pool(name="work", bufs=2) as workp,
            tc.tile_pool(name="stg", bufs=16) as stgp,
            tc.tile_pool(name="lut", bufs=1) as lutp,
            tc.tile_pool(name="outs", bufs=4) as outp,
            tc.tile_pool(name="ps", bufs=8, space="PSUM") as psp,
            tc.tile_pool(name="dram", bufs=1, space="DRAM") as dramp,
        ):
            # ---- constants -------------------------------------------------
            cbdh_sb = constp.tile([128, CK], BF16, tag="cbdh_sb")
            nc.sync.dma_start(out=cbdh_sb[:, :], in_=cbd_h.ap()[:, :])
            cbdl_sb = constp.tile([128, CK], BF16, tag="cbdl_sb")
            nc.sync.dma_start(out=cbdl_sb[:, :], in_=cbd_l.ap()[:, :])
            c2_sb = constp.tile([128, CK], F32, tag="c2_sb")
            nc.sync.dma_start(out=c2_sb[:, :], in_=c2r.ap()[:, :])
            iota_sb = constp.tile([128, 128], F32, tag="iota_sb")
            nc.sync.dma_start(out=iota_sb[:, :], in_=iotar.ap()[:, :])
            identb = constp.tile([128, 128], BF16, tag="identb")
            make_identity(nc, identb[:, :])
            identf = constp.tile([128, 128], F32, tag="identf")
            make_identity(nc, identf[:, :])

            # DRAM scratch for collectives
            kt_bounce = dramp.tile([NC, TOK], BF16, tag="kt_bounce")
            kt_all = dramp.tile([N_CORES * NC, TOK], BF16, tag="kt_all")
            mx_in = dramp.tile([1, 16], F32, tag="mx_in")
            mx_out = dramp.tile([1, 16], F32, tag="mx_out")

            # ---- phase 1: scores (transposed), incremental argmin ----------
            # xc^T[ck, n] via cent-stationary fp32 matmuls (4 pairs col-packed
            # per 128-row chunk), PE-transposed back to score[n, ck].  Each
            # 128-wide chunk holds 8 complete codebooks, so the first-index
            # argmin runs per (chunk, token-tile) right behind the transpose.
            # kminbig[:, 64t + c] = argmin index of (token tile t, codebook c)
            kminbig = workp.tile([128, NT * NC], F32, tag="kminbig", bufs=1)
            for j in range(NCHUNK):
                ps_xct = psp.tile([128, TOK], F32, tag="ps")
                for mcol in range(4):
                    p = 4 * j + mcol
                    xh_t = xtp.tile([128, TOK], BF16, tag="xht")
                    nc.sync.dma_start(out=xh_t[:, :],
                                      in_=xh.ap()[128 * p:128 * (p + 1), :])
                    xl_t = xtp.tile([128, TOK], BF16, tag="xlt")
                    nc.sync.dma_start(out=xl_t[:, :],
                                      in_=xl.ap()[128 * p:128 * (p + 1), :])
                    passes = [(cbdh_sb, xh_t), (cbdh_sb, xl_t),
                              (cbdl_sb, xh_t)]
                    for i, (cb, xt_) in enumerate(passes):
                        nc.tensor.matmul(
                            ps_xct[32 * mcol:32 * (mcol + 1), :],
                            lhsT=cb[:, 32 * p:32 * (p + 1)],
                            rhs=xt_[:, :],
                            start=(i == 0), stop=(i == 2),
                            tile_position=(0, 32 * mcol),
                        )
                xct_sb = xctp.tile([128, TOK], F32, tag="xct")
                nc.vector.tensor_copy(out=xct_sb[:, :], in_=ps_xct[:, :])
                ps_tr4 = psp.tile([128, TOK], F32, tag="ps")
                for t in range(NT):
                    nc.tensor.transpose(
                        ps_tr4[:, 128 * t:128 * (t + 1)],
                        xct_sb[:, 128 * t:128 * (t + 1)],
                        identf[:, :],
                    )
                # batched score + first-index argmin for 4 token tiles at once
                ssl = workp.tile([128, TOK], F32, tag="ssl")
                nc.vector.scalar_tensor_tensor(
                    out=ssl[:, :].rearrange("p (t f) -> p t f", f=128),
                    in0=ps_tr4[:, :].rearrange("p (t f) -> p t f", f=128),
                    in1=c2_sb[:, 128 * j:128 * (j + 1)].unsqueeze(1)
                        .broadcast_to([128, NT, 128]),
                    scalar=-2.0,
                    op0=mybir.AluOpType.mult,
                    op1=mybir.AluOpType.add,
                )
                ssl4 = ssl[:, :].rearrange("p (t c k) -> p t c k", k=K, c=8)
                m32 = workp.tile([128, NT * 8], F32, tag="m32")
                nc.vector.tensor_reduce(
                    out=m32[:, :], in_=ssl4, axis=mybir.AxisListType.X,
                    op=mybir.AluOpType.min,
                )
                eq = workp.tile([128, TOK], F32, tag="eq")
                nc.vector.tensor_tensor(
                    out=eq[:, :].rearrange("p (t c k) -> p t c k", k=K, c=8),
                    in0=ssl4,
                    in1=m32[:, :].rearrange("p (t c) -> p t c", c=8)
                        .unsqueeze(3).broadcast_to([128, NT, 8, K]),
                    op=mybir.AluOpType.is_equal,
                )
                cand = workp.tile([128, TOK], F32, tag="cand")
                nc.vector.scalar_tensor_tensor(
                    out=cand[:, :].rearrange("p (t f) -> p t f", f=128),
                    in0=eq[:, :].rearrange("p (t f) -> p t f", f=128),
                    in1=iota_sb[:, :].unsqueeze(1)
                        .broadcast_to([128, NT, 128]),
                    scalar=-BIG,
                    op0=mybir.AluOpType.mult, op1=mybir.AluOpType.add,
                )
                nc.vector.tensor_reduce(
                    out=kminbig[:, :].rearrange(
                        "p (t c) -> p t c", c=NC)[:, :, 8 * j:8 * (j + 1)],
                    in_=cand[:, :].rearrange("p (t c k) -> p t c k", k=K, c=8),
                    axis=mybir.AxisListType.X, op=mybir.AluOpType.min,
                )

            kt_sb = constp.tile([NC, TOK], BF16, tag="kt_sb")
            for t in range(NT):
                kminb = workp.tile([128, NC], BF16, tag="kminb")
                nc.vector.tensor_scalar_add(
                    kminb[:, :], kminbig[:, NC * t:NC * (t + 1)], BIG)
                # transpose kmin [n, c] -> [c, n] (values 0..15, exact bf16)
                ps_kt = psp.tile([NC, 128], BF16, tag="ps")
                nc.tensor.transpose(ps_kt[:, :], kminb[:, :], identb[:, :])
                nc.vector.tensor_copy(
                    out=kt_sb[:, 128 * t:128 * (t + 1)], in_=ps_kt[:, :])

            # ---- phase 2: AllGather the indices (64KB/core) ----------------
            nc.sync.dma_start(out=kt_bounce[:, :], in_=kt_sb[:, :])
            nc.gpsimd.collective_compute(
                "AllGather",
                mybir.AluOpType.bypass,
                replica_groups=[list(range(N_CORES))],
                ins=[kt_bounce.opt()],
                outs=[kt_all.opt()],
            )

            # ---- phase 3: lut = centroids @ weight ------------------------
            # 3-pass bf16 hi/lo (cbdh*wh + cbdh*wl + cbdl*wh): ~2^-18 per
            # product, bf16 matmul speed, col-packs 4 pairs per PSUM bank.
            lut_sb = lutp.tile([128, NCHUNK * OSH], F32, tag="lut")
            for j in range(NCHUNK):
                ps_lut = psp.tile([128, OSH], F32, tag="ps")
                for mcol in range(4):
                    p = 4 * j + mcol
                    wh_t = wp.tile([128, OSH], BF16, tag="wht")
                    nc.sync.dma_start(
                        out=wh_t[:, :],
                        in_=wsh_h.ap()[128 * p:128 * (p + 1), :])
                    wl_t = wp.tile([128, OSH], BF16, tag="wlt")
                    nc.sync.dma_start(
                        out=wl_t[:, :],
                        in_=wsh_l.ap()[128 * p:128 * (p + 1), :])
                    passes = [(cbdh_sb, wh_t), (cbdh_sb, wl_t),
                              (cbdl_sb, wh_t)]
                    for i, (cb, wt) in enumerate(passes):
                        nc.tensor.matmul(
                            ps_lut[32 * mcol:32 * (mcol + 1), :],
                            lhsT=cb[:, 32 * p:32 * (p + 1)],
                            rhs=wt[:, :],
                            start=(i == 0), stop=(i == 2),
                            tile_position=(0, 32 * mcol),
                        )
                nc.vector.tensor_copy(
                    out=lut_sb[:, OSH * j:OSH * (j + 1)], in_=ps_lut[:, :])

            # ---- phase 4: global absmax -> AllReduce(max) -> scale ---------
            mx8 = constp.tile([128, NCHUNK], F32, tag="mx8")
            for j in range(NCHUNK):
                nc.vector.tensor_reduce(
                    out=mx8[:, j:j + 1],
                    in_=lut_sb[:, OSH * j:OSH * (j + 1)],
                    axis=mybir.AxisListType.X, op=mybir.AluOpType.max,
                    apply_absolute_value=True,
                )
            mxl = constp.tile([128, 1], F32, tag="mxl")
            nc.vector.tensor_reduce(
                out=mxl[:, :], in_=mx8[:, :], axis=mybir.AxisListType.X,
                op=mybir.AluOpType.max,
            )
            mxp = constp.tile([128, 1], F32, tag="mxp")
            nc.gpsimd.partition_all_reduce(
                mxp[:, :], mxl[:, :], channels=128,
                reduce_op=bass_isa.ReduceOp.max,
            )
            zrow = constp.tile([1, 16], F32, tag="zrow")
            nc.vector.memset(zrow[:, :], 0.0)
            nc.vector.tensor_copy(out=zrow[0:1, 0:1], in_=mxp[0:1, 0:1])
            nc.sync.dma_start(out=mx_in[:, :], in_=zrow[:, :])
            nc.gpsimd.collective_compute(
                "AllReduce",
                mybir.AluOpType.max,
                replica_groups=[list(range(N_CORES))],
                ins=[mx_in.opt()],
                outs=[mx_out.opt()],
            )
            gmax_row = constp.tile([1, 16], F32, tag="gmax_row")
            nc.sync.dma_start(out=gmax_row[:, :], in_=mx_out[:, :])
            gmax = constp.tile([128, 1], F32, tag="gmax")
            nc.gpsimd.partition_broadcast(gmax[:, :], gmax_row[0:1, 0:1])
            # s = gmax/127 and 1/s via reciprocal (DVE has no divide); the
            # <=2ulp drift is far inside the error budget.
            s_col = constp.tile([128, 1], F32, tag="s_col")
            nc.vector.tensor_scalar(
                out=s_col[:, :], in0=gmax[:, :],
                scalar1=float(np.float32(1.0) / np.float32(127.0)),
                scalar2=None, op0=mybir.AluOpType.mult,
            )
            rgmax = constp.tile([128, 1], F32, tag="rgmax")
            nc.vector.reciprocal(rgmax[:, :], gmax[:, :])
            inv_s = constp.tile([128, 1], F32, tag="inv_s")
            nc.vector.tensor_scalar(
                out=inv_s[:, :], in0=rgmax[:, :], scalar1=127.0, scalar2=None,
                op0=mybir.AluOpType.mult,
            )

            # ---- phase 5: quantize-dequantize lut -> bf16 ------------------
            lutdq = lutp.tile([128, NCHUNK * OSH], BF16, tag="lutdq")
            for j in range(NCHUNK):
                qm = wp.tile([128, OSH], F32, tag="qm")
                nc.vector.tensor_scalar(
                    out=qm[:, :], in0=lut_sb[:, OSH * j:OSH * (j + 1)],
                    scalar1=inv_s[:, 0:1], scalar2=MAGIC,
                    op0=mybir.AluOpType.mult, op1=mybir.AluOpType.add,
                )
                nc.vector.tensor_scalar(
                    out=lutdq[:, OSH * j:OSH * (j + 1)], in0=qm[:, :],
                    scalar1=MAGIC, scalar2=s_col[:, 0:1],
                    op0=mybir.AluOpType.subtract, op1=mybir.AluOpType.mult,
                )

            bias_sb = constp.tile([128, OSH], F32, tag="bias_sb")
            nc.sync.dma_start(out=bias_sb[:, :], in_=biasr.ap()[:, :])
            emat_sb = constp.tile([NC, CK], BF16, tag="emat_sb")
            nc.sync.dma_start(out=emat_sb[:, :], in_=emat.ap()[:, :])
            kcol_sb = constp.tile([128, 1], F32, tag="kcol_sb")
            nc.sync.dma_start(out=kcol_sb[:, :], in_=kcol.ap()[:, :])

            # ---- phase 6: expand one-hots + gather matmul ------------------
            for r in range(N_CORES):
                ktr = stgp.tile([NC, TOK], BF16, tag="ktr", bufs=4)
                nc.sync.dma_start(
                    out=ktr[:, :],
                    in_=kt_all[NC * r:NC * (r + 1), :])
                stg_tiles = []
                for j in range(NCHUNK):
                    # replicate kmin rows onto the 16 k-partitions of chunk j
                    ps_rep = psp.tile([128, TOK], F32, tag="ps")
                    nc.tensor.matmul(
                        ps_rep[:, :],
                        lhsT=emat_sb[:, 128 * j:128 * (j + 1)],
                        rhs=ktr[:, :],
                        start=True, stop=True,
                    )
                    stg_t = stgp.tile([128, TOK], BF16, tag="stg")
                    nc.vector.tensor_scalar(
                        out=stg_t[:, :], in0=ps_rep[:, :],
                        scalar1=kcol_sb[:, 0:1], scalar2=None,
                        op0=mybir.AluOpType.is_equal,
                    )
                    stg_tiles.append(stg_t)
                for t in range(NT):
                    ps_o = psp.tile([128, OSH], F32, tag="ps")
                    for j in range(NCHUNK):
                        nc.tensor.matmul(
                            ps_o[:, :],
                            lhsT=stg_tiles[j][:, 128 * t:128 * (t + 1)],
                            rhs=lutdq[:, OSH * j:OSH * (j + 1)],
                            start=(j == 0), stop=(j == NCHUNK - 1),
                        )
                    o_sb = outp.tile([128, OSH], BF16, tag="outsb")
                    nc.vector.tensor_tensor(
                        out=o_sb[:, :], in0=ps_o[:, :], in1=bias_sb[:, :],
                        op=mybir.AluOpType.add,
                    )
                    row0 = TOK * r + 128 * t
                    nc.sync.dma_start(
                        out=out.ap()[row0:row0 + 128, :], in_=o_sb[:, :])

    nc.compile()
    return nc


def _prep_inputs(x, centroids, weight, bias):
    import ml_dtypes

    x = np.ascontiguousarray(np.asarray(x, dtype=np.float32)).reshape(BN, IN_F)
    cent = np.asarray(centroids, dtype=np.float32)
    w = np.asarray(weight, dtype=np.float32)
    bias = np.asarray(bias, dtype=np.float32)

    c2 = (cent ** 2).sum(axis=-1).reshape(CK)  # [1024] flat (c,k)
    c2r = np.ascontiguousarray(np.broadcast_to(c2, (128, CK)))
    iota = np.tile(np.arange(K, dtype=np.float32), 8)
    iotar = np.ascontiguousarray(np.broadcast_to(iota, (128, 128)))
    cbd = np.zeros((128, CK), np.float32)
    for p in range(NPAIR):
        cbd[0:SUBV, 32 * p:32 * p + K] = cent[2 * p].T
        cbd[SUBV:128, 32 * p + K:32 * p + 2 * K] = cent[2 * p + 1].T
    cbd_h = cbd.astype(ml_dtypes.bfloat16)
    cbd_l = (cbd - cbd_h.astype(np.float32)).astype(ml_dtypes.bfloat16)
    # E[c, ck] = 1 where ck // 16 == c  (replication matrix)
    emat = (np.arange(CK)[None, :] // K == np.arange(NC)[:, None]).astype(
        ml_dtypes.bfloat16)
    kcol = np.ascontiguousarray(
        (np.arange(128, dtype=np.float32) % K).reshape(128, 1))

    in_maps = []
    for r in range(N_CORES):
        xT_r = np.ascontiguousarray(x[TOK * r:TOK * (r + 1)].T)
        xh_r = xT_r.astype(ml_dtypes.bfloat16)
        xl_r = (xT_r - xh_r.astype(np.float32)).astype(ml_dtypes.bfloat16)
        w_r = np.ascontiguousarray(w[:, :, OSH * r:OSH * (r + 1)]).reshape(
            IN_F, OSH)
        w_h = w_r.astype(ml_dtypes.bfloat16)
        w_l = (w_r - w_h.astype(np.float32)).astype(ml_dtypes.bfloat16)
        bias_r = np.ascontiguousarray(
            np.broadcast_to(bias[OSH * r:OSH * (r + 1)], (128, OSH)))
        in_maps.append({
            "xh": xh_r, "xl": xl_r, "cbd": cbd, "c2r": c2r, "iotar": iotar,
            "wsh_h": w_h, "wsh_l": w_l, "cbd_h": cbd_h, "cbd_l": cbd_l,
            "biasr": bias_r, "emat": emat, "kcol": kcol,
        })
    return in_maps


def kernel(x, centroids, weight, inverse_temperature_logit, bias,
           **_unused) -> np.ndarray:
    if "nc" not in _CACHE:
        _CACHE["nc"] = _build()
    nc = _CACHE["nc"]
    in_maps = _prep_inputs(x, centroids, weight, bias)
    res = run_bass_kernel_spmd(nc, in_maps, core_ids=list(range(N_CORES)))
    out = np.concatenate(
        [res.results[r]["out"].astype(np.float32) for r in range(N_CORES)],
        axis=1)
    return out.reshape(2, BN // 2, OUT_F)


# revision 13
# speedup vs baseline: 1.1986x; 1.0227x over previous
"""AMMLinear (VQ codebook) forward on 8 TRN2 NeuronCores.

The straight-through estimator makes the forward VALUE exactly
    out[n, o] = sum_c lut_dq[c, argmin_k dist(x_cn, cent_ck), o] + bias[o]
with lut = centroids @ weight (per codebook) and lut_dq a global-scale int8
quantize-dequantize of lut.  The softmax/attention path only shapes gradients.

Sharding: tokens (BN=4096 -> 512/core) for the score/argmin phase, output
features (4096 -> 512/core) for the lut/gather phase.  One AllGather moves the
bf16 argmin indices (64KB/core), one AllReduce-max the quantization scale.
Every core then expands all 4096 tokens' one-hot codes locally (replication
DMA + is_equal) and computes its o-shard of the gather matmul.
Output is assembled host-side by concatenating the per-core o-shards.

Numerics: scores and lut are computed as 3-pass bf16 hi/lo matmuls
(a*b ~ ah*bh + ah*bl + al*bh, fp32 PSUM accumulate, ~2^-18/product), the
gather matmul in bf16 (exact one-hots, bf16-rounded lut_dq), output in bf16.
Measured end-to-end rel err ~5e-3 against the fp32 reference.
"""

import numpy as np

import concourse.bass as bass
import concourse.mybir as mybir
import concourse.tile as tile
import concourse.bass_isa as bass_isa
from concourse import bacc
from concourse.bass_utils import run_bass_kernel_spmd
from concourse.masks import make_identity

F32 = mybir.dt.float32
BF16 = mybir.dt.bfloat16

N_CORES = 8
NC, K, IN_F, OUT_F = 64, 16, 4096, 4096
SUBV = IN_F // NC          # 64
BN = 4096                  # 2*2048 tokens
TOK = BN // N_CORES        # 512 tokens per core
NT = TOK // 128            # 4 token tiles per core
NPAIR = NC // 2            # 32 codebook pairs
CK = NC * K                # 1024 (codebook,centroid) flat index
NCHUNK = CK // 128         # 8 contraction chunks
OSH = OUT_F // N_CORES     # 512 out features per core
MAGIC = 12582912.0         # 1.5 * 2**23: fp32 round-to-nearest-even trick
BIG = 1024.0

_CACHE = {}


def _build():
    nc = bacc.Bacc("TRN2", target_bir_lowering=False, debug=False,
                   num_devices=N_CORES)

    # xhl[:, 0:TOK] = bf16 hi of x^T shard, [:, TOK:2*TOK] = bf16 lo
    xhl = nc.declare_dram_parameter("xhl", [IN_F, 2 * TOK], BF16,
                                    isOutput=False)
    # whl[:, 0:OSH] = bf16 hi of weight o-shard, [:, OSH:2*OSH] = lo
    whl = nc.declare_dram_parameter("whl", [IN_F, 2 * OSH], BF16,
                                    isOutput=False)
    cbd_h = nc.declare_dram_parameter("cbd_h", [128, CK], BF16, isOutput=False)
    cbd_l = nc.declare_dram_parameter("cbd_l", [128, CK], BF16, isOutput=False)
    c2r = nc.declare_dram_parameter("c2r", [128, CK], F32, isOutput=False)
    iotar = nc.declare_dram_parameter("iotar", [128, 128], F32, isOutput=False)
    biasr = nc.declare_dram_parameter("biasr", [128, OSH], F32, isOutput=False)
    kcol = nc.declare_dram_parameter("kcol", [128, 1], F32, isOutput=False)
    out = nc.declare_dram_parameter("out", [BN, OSH], BF16, isOutput=True)

    with tile.TileContext(nc) as tc:
        with (
            tc.tile_pool(name="consts", bufs=1) as constp,
            tc.tile_pool(name="xt", bufs=6) as xtp,
            tc.tile_pool(name="wt", bufs=6) as wp,
            tc.tile_pool(name="xct", bufs=3) as xctp,
            tc.tile_pool(name="work", bufs=2) as workp,
            tc.tile_pool(name="stg", bufs=16) as stgp,
            tc.tile_pool(name="strep", bufs=8) as strepp,
            tc.tile_pool(name="lut", bufs=1) as lutp,
            tc.tile_pool(name="outs", bufs=2) as outp,
            tc.tile_pool(name="ps", bufs=8, space="PSUM") as psp,
            tc.tile_pool(name="dram", bufs=1, space="DRAM") as dramp,
        ):
            # ---- constants -------------------------------------------------
            cbdh_sb = constp.tile([128, CK], BF16, tag="cbdh_sb")
            nc.sync.dma_start(out=cbdh_sb[:, :], in_=cbd_h.ap()[:, :])
            cbdl_sb = constp.tile([128, CK], BF16, tag="cbdl_sb")
            nc.sync.dma_start(out=cbdl_sb[:, :], in_=cbd_l.ap()[:, :])
            c2_sb = constp.tile([128, CK], F32, tag="c2_sb")
            nc.sync.dma_start(out=c2_sb[:, :], in_=c2r.ap()[:, :])
            iota_sb = constp.tile([128, 128], F32, tag="iota_sb")
            nc.sync.dma_start(out=iota_sb[:, :], in_=iotar.ap()[:, :])
            identb = constp.tile([128, 128], BF16, tag="identb")
            make_identity(nc, identb[:, :])
            identf = constp.tile([128, 128], F32, tag="identf")
            make_identity(nc, identf[:, :])

            # DRAM scratch for collectives
            kt_bounce = dramp.tile([NC, TOK], BF16, tag="kt_bounce")
            kt_all = dramp.tile([N_CORES * NC, TOK], BF16, tag="kt_all")
            mx_in = dramp.tile([1, 16], F32, tag="mx_in")
            mx_out = dramp.tile([1, 16], F32, tag="mx_out")

            # ---- phase 1+3 interleaved: scores+argmin and lut --------------
            # Scores: xc^T[ck, n] = cbd^T x (3-pass bf16 hi/lo, 4 pairs
            # col-packed per chunk), PE-transposed back to [n, ck]; each
            # chunk holds 8 complete codebooks so the first-index argmin
            # runs right behind the transposes, batched over 4 token tiles.
            # Lut: same 3-pass structure against the weight o-shard.
            kminbig = workp.tile([128, NT * NC], F32, tag="kminbig", bufs=1)
            lut_sb = lutp.tile([128, NCHUNK * OSH], F32, tag="lut")
            mx8 = constp.tile([128, NCHUNK], F32, tag="mx8")
            for j in range(NCHUNK):
                # --- score chunk j ---
                ps_xct = psp.tile([128, TOK], F32, tag="ps")
                for mcol in range(4):
                    p = 4 * j + mcol
                    xt_t = xtp.tile([128, 2 * TOK], BF16, tag="xt")
                    nc.sync.dma_start(out=xt_t[:, :],
                                      in_=xhl.ap()[128 * p:128 * (p + 1), :])
                    passes = [(cbdh_sb, 0), (cbdh_sb, TOK), (cbdl_sb, 0)]
                    for i, (cb, off) in enumerate(passes):
                        nc.tensor.matmul(
                            ps_xct[32 * mcol:32 * (mcol + 1), :],
                            lhsT=cb[:, 32 * p:32 * (p + 1)],
                            rhs=xt_t[:, off:off + TOK],
                            start=(i == 0), stop=(i == 2),
                            tile_position=(0, 32 * mcol),
                        )
                xct_sb = xctp.tile([128, TOK], F32, tag="xct")
                nc.vector.tensor_copy(out=xct_sb[:, :], in_=ps_xct[:, :])
                ps_tr4 = psp.tile([128, TOK], F32, tag="ps")
                for t in range(NT):
                    nc.tensor.transpose(
                        ps_tr4[:, 128 * t:128 * (t + 1)],
                        xct_sb[:, 128 * t:128 * (t + 1)],
                        identf[:, :],
                    )
                # --- lut chunk j ---
                ps_lut = psp.tile([128, OSH], F32, tag="ps")
                for mcol in range(4):
                    p = 4 * j + mcol
                    w_t = wp.tile([128, 2 * OSH], BF16, tag="wt")
                    nc.sync.dma_start(
                        out=w_t[:, :],
                        in_=whl.ap()[128 * p:128 * (p + 1), :])
                    passes = [(cbdh_sb, 0), (cbdh_sb, OSH), (cbdl_sb, 0)]
                    for i, (cb, off) in enumerate(passes):
                        nc.tensor.matmul(
                            ps_lut[32 * mcol:32 * (mcol + 1), :],
                            lhsT=cb[:, 32 * p:32 * (p + 1)],
                            rhs=w_t[:, off:off + OSH],
                            start=(i == 0), stop=(i == 2),
                            tile_position=(0, 32 * mcol),
                        )
                nc.vector.tensor_copy(
                    out=lut_sb[:, OSH * j:OSH * (j + 1)], in_=ps_lut[:, :])
                nc.vector.tensor_reduce(
                    out=mx8[:, j:j + 1],
                    in_=lut_sb[:, OSH * j:OSH * (j + 1)],
                    axis=mybir.AxisListType.X, op=mybir.AluOpType.max,
                    apply_absolute_value=True,
                )
                # --- batched score + first-index argmin (4 token tiles) ---
                ssl = workp.tile([128, TOK], F32, tag="ssl")
                nc.vector.scalar_tensor_tensor(
                    out=ssl[:, :].rearrange("p (t f) -> p t f", f=128),
                    in0=ps_tr4[:, :].rearrange("p (t f) -> p t f", f=128),
                    in1=c2_sb[:, 128 * j:128 * (j + 1)].unsqueeze(1)
                        .broadcast_to([128, NT, 128]),
                    scalar=-2.0,
                    op0=mybir.AluOpType.mult,
                    op1=mybir.AluOpType.add,
                )
                ssl4 = ssl[:, :].rearrange("p (t c k) -> p t c k", k=K, c=8)
                m32 = workp.tile([128, NT * 8], F32, tag="m32")
                nc.vector.tensor_reduce(
                    out=m32[:, :], in_=ssl4, axis=mybir.AxisListType.X,
                    op=mybir.AluOpType.min,
                )
                eq = workp.tile([128, TOK], F32, tag="eq")
                nc.vector.tensor_tensor(
                    out=eq[:, :].rearrange("p (t c k) -> p t c k", k=K, c=8),
                    in0=ssl4,
                    in1=m32[:, :].rearrange("p (t c) -> p t c", c=8)
                        .unsqueeze(3).broadcast_to([128, NT, 8, K]),
                    op=mybir.AluOpType.is_equal,
                )
                cand = workp.tile([128, TOK], F32, tag="cand")
                nc.vector.scalar_tensor_tensor(
                    out=cand[:, :].rearrange("p (t f) -> p t f", f=128),
                    in0=eq[:, :].rearrange("p (t f) -> p t f", f=128),
                    in1=iota_sb[:, :].unsqueeze(1)
                        .broadcast_to([128, NT, 128]),
                    scalar=-BIG,
                    op0=mybir.AluOpType.mult, op1=mybir.AluOpType.add,
                )
                nc.vector.tensor_reduce(
                    out=kminbig[:, :].rearrange(
                        "p (t c) -> p t c", c=NC)[:, :, 8 * j:8 * (j + 1)],
                    in_=cand[:, :].rearrange("p (t c k) -> p t c k", k=K, c=8),
                    axis=mybir.AxisListType.X, op=mybir.AluOpType.min,
                )

            # ---- phase 2: AllGather the indices (64KB/core) ----------------
            kt_sb = constp.tile([NC, TOK], BF16, tag="kt_sb")
            for t in range(NT):
                kminb = workp.tile([128, NC], BF16, tag="kminb")
                nc.vector.tensor_scalar_add(
                    kminb[:, :], kminbig[:, NC * t:NC * (t + 1)], BIG)
                # transpose kmin [n, c] -> [c, n] (values 0..15, exact bf16)
                ps_kt = psp.tile([NC, 128], BF16, tag="ps")
                nc.tensor.transpose(ps_kt[:, :], kminb[:, :], identb[:, :])
                nc.vector.tensor_copy(
                    out=kt_sb[:, 128 * t:128 * (t + 1)], in_=ps_kt[:, :])
            nc.sync.dma_start(out=kt_bounce[:, :], in_=kt_sb[:, :])
            nc.gpsimd.collective_compute(
                "AllGather",
                mybir.AluOpType.bypass,
                replica_groups=[list(range(N_CORES))],
                ins=[kt_bounce.opt()],
                outs=[kt_all.opt()],
            )

            # ---- phase 4: global absmax -> AllReduce(max) -> scale ---------
            mxl = constp.tile([128, 1], F32, tag="mxl")
            nc.vector.tensor_reduce(
                out=mxl[:, :], in_=mx8[:, :], axis=mybir.AxisListType.X,
                op=mybir.AluOpType.max,
            )
            mxp = constp.tile([128, 1], F32, tag="mxp")
            nc.gpsimd.partition_all_reduce(
                mxp[:, :], mxl[:, :], channels=128,
                reduce_op=bass_isa.ReduceOp.max,
            )
            zrow = constp.tile([1, 16], F32, tag="zrow")
            nc.vector.memset(zrow[:, :], 0.0)
            nc.vector.tensor_copy(out=zrow[0:1, 0:1], in_=mxp[0:1, 0:1])
            nc.sync.dma_start(out=mx_in[:, :], in_=zrow[:, :])
            nc.gpsimd.collective_compute(
                "AllReduce",
                mybir.AluOpType.max,
                replica_groups=[list(range(N_CORES))],
                ins=[mx_in.opt()],
                outs=[mx_out.opt()],
            )
            gmax_row = constp.tile([1, 16], F32, tag="gmax_row")
            nc.sync.dma_start(out=gmax_row[:, :], in_=mx_out[:, :])
            gmax = constp.tile([128, 1], F32, tag="gmax")
            nc.gpsimd.partition_broadcast(gmax[:, :], gmax_row[0:1, 0:1])
            # s = gmax/127 and 1/s via reciprocal (DVE has no divide); the
            # <=2ulp drift is far inside the error budget.
            s_col = constp.tile([128, 1], F32, tag="s_col")
            nc.vector.tensor_scalar(
                out=s_col[:, :], in0=gmax[:, :],
                scalar1=float(np.float32(1.0) / np.float32(127.0)),
                scalar2=None, op0=mybir.AluOpType.mult,
            )
            rgmax = constp.tile([128, 1], F32, tag="rgmax")
            nc.vector.reciprocal(rgmax[:, :], gmax[:, :])
            inv_s = constp.tile([128, 1], F32, tag="inv_s")
            nc.vector.tensor_scalar(
                out=inv_s[:, :], in0=rgmax[:, :], scalar1=127.0, scalar2=None,
                op0=mybir.AluOpType.mult,
            )

            # ---- phase 5: quantize-dequantize lut -> bf16 ------------------
            lutdq = lutp.tile([128, NCHUNK * OSH], BF16, tag="lutdq")
            for j in range(NCHUNK):
                qm = wp.tile([128, OSH], F32, tag="qm")
                nc.vector.tensor_scalar(
                    out=qm[:, :], in0=lut_sb[:, OSH * j:OSH * (j + 1)],
                    scalar1=inv_s[:, 0:1], scalar2=MAGIC,
                    op0=mybir.AluOpType.mult, op1=mybir.AluOpType.add,
                )
                nc.vector.tensor_scalar(
                    out=lutdq[:, OSH * j:OSH * (j + 1)], in0=qm[:, :],
                    scalar1=MAGIC, scalar2=s_col[:, 0:1],
                    op0=mybir.AluOpType.subtract, op1=mybir.AluOpType.mult,
                )

            bias_sb = constp.tile([128, OSH], F32, tag="bias_sb")
            nc.sync.dma_start(out=bias_sb[:, :], in_=biasr.ap()[:, :])
            kcol_sb = constp.tile([128, 1], F32, tag="kcol_sb")
            nc.sync.dma_start(out=kcol_sb[:, :], in_=kcol.ap()[:, :])

            # ---- phase 6: expand one-hots + gather matmul ------------------
            for r in range(N_CORES):
                stg_tiles = []
                for j in range(NCHUNK):
                    # replicate each codebook's index row onto its 16
                    # k-partitions straight out of DRAM (step-0 broadcast)
                    rep = strepp.tile([128, TOK], BF16, tag="strep")
                    nc.sync.dma_start(
                        out=rep[:, :],
                        in_=kt_all[NC * r + 8 * j:NC * r + 8 * (j + 1), :]
                            .unsqueeze(1).broadcast_to([8, K, TOK]),
                    )
                    stg_t = stgp.tile([128, TOK], BF16, tag="stg")
                    nc.vector.tensor_scalar(
                        out=stg_t[:, :], in0=rep[:, :],
                        scalar1=kcol_sb[:, 0:1], scalar2=None,
                        op0=mybir.AluOpType.is_equal,
                    )
                    stg_tiles.append(stg_t)
                o_sb = outp.tile([128, NT * OSH], BF16, tag="outsb")
                for t in range(NT):
                    ps_o = psp.tile([128, OSH], F32, tag="ps")
                    for j in range(NCHUNK):
                        nc.tensor.matmul(
                            ps_o[:, :],
                            lhsT=stg_tiles[j][:, 128 * t:128 * (t + 1)],
                            rhs=lutdq[:, OSH * j:OSH * (j + 1)],
                            start=(j == 0), stop=(j == NCHUNK - 1),
                        )
                    nc.vector.tensor_tensor(
                        out=o_sb[:, OSH * t:OSH * (t + 1)], in0=ps_o[:, :],
                        in1=bias_sb[:, :],
                        op=mybir.AluOpType.add,
                    )
                nc.sync.dma_start(
                    out=out.ap()[TOK * r:TOK * (r + 1), :]
                        .rearrange("(t p) o -> p t o", p=128),
                    in_=o_sb[:, :].rearrange("p (t o) -> p t o", o=OSH),
                )

    nc.compile()
    return nc


def _prep_inputs(x, centroids, weight, bias):
    import ml_dtypes

    x = np.ascontiguousarray(np.asarray(x, dtype=np.float32)).reshape(BN, IN_F)
    cent = np.asarray(centroids, dtype=np.float32)
    w = np.asarray(weight, dtype=np.float32)
    bias = np.asarray(bias, dtype=np.float32)

    c2 = (cent ** 2).sum(axis=-1).reshape(CK)  # [1024] flat (c,k)
    c2r = np.ascontiguousarray(np.broadcast_to(c2, (128, CK)))
    iota = np.tile(np.arange(K, dtype=np.float32), 8)
    iotar = np.ascontiguousarray(np.broadcast_to(iota, (128, 128)))
    cbd = np.zeros((128, CK), np.float32)
    for p in range(NPAIR):
        cbd[0:SUBV, 32 * p:32 * p + K] = cent[2 * p].T
        cbd[SUBV:128, 32 * p + K:32 * p + 2 * K] = cent[2 * p + 1].T
    cbd_h = cbd.astype(ml_dtypes.bfloat16)
    cbd_l = (cbd - cbd_h.astype(np.float32)).astype(ml_dtypes.bfloat16)
    kcol = np.ascontiguousarray(
        (np.arange(128, dtype=np.float32) % K).reshape(128, 1))

    in_maps = []
    for r in range(N_CORES):
        xT_r = np.ascontiguousarray(x[TOK * r:TOK * (r + 1)].T)
        xh_r = xT_r.astype(ml_dtypes.bfloat16)
        xl_r = (xT_r - xh_r.astype(np.float32)).astype(ml_dtypes.bfloat16)
        xhl_r = np.ascontiguousarray(np.concatenate([xh_r, xl_r], axis=1))
        w_r = np.ascontiguousarray(w[:, :, OSH * r:OSH * (r + 1)]).reshape(
            IN_F, OSH)
        w_h = w_r.astype(ml_dtypes.bfloat16)
        w_l = (w_r - w_h.astype(np.float32)).astype(ml_dtypes.bfloat16)
        whl_r = np.ascontiguousarray(np.concatenate([w_h, w_l], axis=1))
        bias_r = np.ascontiguousarray(
            np.broadcast_to(bias[OSH * r:OSH * (r + 1)], (128, OSH)))
        in_maps.append({
            "xhl": xhl_r, "whl": whl_r, "cbd_h": cbd_h, "cbd_l": cbd_l,
            "c2r": c2r, "iotar": iotar, "biasr": bias_r, "kcol": kcol,
        })
    return in_maps


def kernel(x, centroids, weight, inverse_temperature_logit, bias,
           **_unused) -> np.ndarray:
    if "nc" not in _CACHE:
        _CACHE["nc"] = _build()
    nc = _CACHE["nc"]
    in_maps = _prep_inputs(x, centroids, weight, bias)
    res = run_bass_kernel_spmd(nc, in_maps, core_ids=list(range(N_CORES)))
    out = np.concatenate(
        [res.results[r]["out"].astype(np.float32) for r in range(N_CORES)],
        axis=1)
    return out.reshape(2, BN // 2, OUT_F)


# revision 14
# speedup vs baseline: 1.2687x; 1.0585x over previous
"""AMMLinear (VQ codebook) forward on 8 TRN2 NeuronCores.

The straight-through estimator makes the forward VALUE exactly
    out[n, o] = sum_c lut_dq[c, argmin_k dist(x_cn, cent_ck), o] + bias[o]
with lut = centroids @ weight (per codebook) and lut_dq a global-scale int8
quantize-dequantize of lut.  The softmax/attention path only shapes gradients.

Sharding: tokens (BN=4096 -> 512/core) for the score/argmin phase, output
features (4096 -> 512/core) for the lut/gather phase.  One AllGather moves the
bf16 argmin indices (64KB/core), one AllReduce-max the quantization scale.
Every core then expands all 4096 tokens' one-hot codes locally (replication
DMA + is_equal) and computes its o-shard of the gather matmul.
Output is assembled host-side by concatenating the per-core o-shards.

Numerics: scores and lut are computed as 3-pass bf16 hi/lo matmuls
(a*b ~ ah*bh + ah*bl + al*bh, fp32 PSUM accumulate, ~2^-18/product), the
gather matmul in bf16 (exact one-hots, bf16-rounded lut_dq), output in bf16.
Measured end-to-end rel err ~5e-3 against the fp32 reference.
"""

import numpy as np

import concourse.bass as bass
import concourse.mybir as mybir
import concourse.tile as tile
import concourse.bass_isa as bass_isa
from concourse import bacc
from concourse.bass_utils import run_bass_kernel_spmd
from concourse.masks import make_identity

F32 = mybir.dt.float32
BF16 = mybir.dt.bfloat16

N_CORES = 8
NC, K, IN_F, OUT_F = 64, 16, 4096, 4096
SUBV = IN_F // NC          # 64
BN = 4096                  # 2*2048 tokens
TOK = BN // N_CORES        # 512 tokens per core
NT = TOK // 128            # 4 token tiles per core
NPAIR = NC // 2            # 32 codebook pairs
CK = NC * K                # 1024 (codebook,centroid) flat index
NCHUNK = CK // 128         # 8 contraction chunks
OSH = OUT_F // N_CORES     # 512 out features per core
MAGIC = 12582912.0         # 1.5 * 2**23: fp32 round-to-nearest-even trick
BIG = 1024.0

_CACHE = {}


def _build():
    nc = bacc.Bacc("TRN2", target_bir_lowering=False, debug=False,
                   num_devices=N_CORES)

    # xhl[:, 0:TOK] = bf16 hi of x^T shard, [:, TOK:2*TOK] = bf16 lo
    xhl = nc.declare_dram_parameter("xhl", [IN_F, 2 * TOK], BF16,
                                    isOutput=False)
    # whl[:, 0:OSH] = bf16 hi of weight o-shard, [:, OSH:2*OSH] = lo
    whl = nc.declare_dram_parameter("whl", [IN_F, 2 * OSH], BF16,
                                    isOutput=False)
    cbd_h = nc.declare_dram_parameter("cbd_h", [128, CK], BF16, isOutput=False)
    cbd_l = nc.declare_dram_parameter("cbd_l", [128, CK], BF16, isOutput=False)
    c2r = nc.declare_dram_parameter("c2r", [128, CK], F32, isOutput=False)
    iotar = nc.declare_dram_parameter("iotar", [128, 128], F32, isOutput=False)
    biasr = nc.declare_dram_parameter("biasr", [128, OSH], F32, isOutput=False)
    kcol = nc.declare_dram_parameter("kcol", [128, 1], F32, isOutput=False)
    out = nc.declare_dram_parameter("out", [BN, OSH], BF16, isOutput=True)

    with tile.TileContext(nc) as tc:
        with (
            tc.tile_pool(name="consts", bufs=1) as constp,
            tc.tile_pool(name="xt", bufs=6) as xtp,
            tc.tile_pool(name="wt", bufs=6) as wp,
            tc.tile_pool(name="xct", bufs=3) as xctp,
            tc.tile_pool(name="work", bufs=2) as workp,
            tc.tile_pool(name="stg", bufs=16) as stgp,
            tc.tile_pool(name="strep", bufs=8) as strepp,
            tc.tile_pool(name="lut", bufs=1) as lutp,
            tc.tile_pool(name="outs", bufs=2) as outp,
            tc.tile_pool(name="ps", bufs=8, space="PSUM") as psp,
            tc.tile_pool(name="dram", bufs=1, space="DRAM") as dramp,
        ):
            # ---- constants -------------------------------------------------
            cbdh_sb = constp.tile([128, CK], BF16, tag="cbdh_sb")
            nc.sync.dma_start(out=cbdh_sb[:, :], in_=cbd_h.ap()[:, :])
            cbdl_sb = constp.tile([128, CK], BF16, tag="cbdl_sb")
            nc.sync.dma_start(out=cbdl_sb[:, :], in_=cbd_l.ap()[:, :])
            c2_sb = constp.tile([128, CK], F32, tag="c2_sb")
            nc.sync.dma_start(out=c2_sb[:, :], in_=c2r.ap()[:, :])
            iota_sb = constp.tile([128, 128], F32, tag="iota_sb")
            nc.sync.dma_start(out=iota_sb[:, :], in_=iotar.ap()[:, :])
            identb = constp.tile([128, 128], BF16, tag="identb")
            make_identity(nc, identb[:, :])
            identf = constp.tile([128, 128], F32, tag="identf")
            make_identity(nc, identf[:, :])

            # DRAM scratch for collectives
            kt_bounce = dramp.tile([NC, TOK], BF16, tag="kt_bounce")
            kt_all = dramp.tile([N_CORES * NC, TOK], BF16, tag="kt_all")
            mx_in = dramp.tile([1, 16], F32, tag="mx_in")
            mx_out = dramp.tile([1, 16], F32, tag="mx_out")

            # ---- phase A: lut = centroids @ weight (3-pass bf16 hi/lo) ----
            # Runs first so the absmax -> AllReduce(max) chain is issued as
            # early as possible (the collective queue is FIFO: the first
            # collective blocks the second until it completes).
            lut_sb = lutp.tile([128, NCHUNK * OSH], F32, tag="lut")
            mx8 = constp.tile([128, NCHUNK], F32, tag="mx8")
            for j in range(NCHUNK):
                ps_lut = psp.tile([128, OSH], F32, tag="ps")
                for mcol in range(4):
                    p = 4 * j + mcol
                    w_t = wp.tile([128, 2 * OSH], BF16, tag="wt")
                    nc.sync.dma_start(
                        out=w_t[:, :],
                        in_=whl.ap()[128 * p:128 * (p + 1), :])
                    passes = [(cbdh_sb, 0), (cbdh_sb, OSH), (cbdl_sb, 0)]
                    for i, (cb, off) in enumerate(passes):
                        nc.tensor.matmul(
                            ps_lut[32 * mcol:32 * (mcol + 1), :],
                            lhsT=cb[:, 32 * p:32 * (p + 1)],
                            rhs=w_t[:, off:off + OSH],
                            start=(i == 0), stop=(i == 2),
                            tile_position=(0, 32 * mcol),
                        )
                nc.vector.tensor_copy(
                    out=lut_sb[:, OSH * j:OSH * (j + 1)], in_=ps_lut[:, :])
                nc.vector.tensor_reduce(
                    out=mx8[:, j:j + 1],
                    in_=lut_sb[:, OSH * j:OSH * (j + 1)],
                    axis=mybir.AxisListType.X, op=mybir.AluOpType.max,
                    apply_absolute_value=True,
                )

            # ---- phase B: global absmax -> AllReduce(max) -> scale ---------
            mxl = constp.tile([128, 1], F32, tag="mxl")
            nc.vector.tensor_reduce(
                out=mxl[:, :], in_=mx8[:, :], axis=mybir.AxisListType.X,
                op=mybir.AluOpType.max,
            )
            mxp = constp.tile([128, 1], F32, tag="mxp")
            nc.gpsimd.partition_all_reduce(
                mxp[:, :], mxl[:, :], channels=128,
                reduce_op=bass_isa.ReduceOp.max,
            )
            zrow = constp.tile([1, 16], F32, tag="zrow")
            nc.vector.memset(zrow[:, :], 0.0)
            nc.vector.tensor_copy(out=zrow[0:1, 0:1], in_=mxp[0:1, 0:1])
            nc.sync.dma_start(out=mx_in[:, :], in_=zrow[:, :])
            nc.gpsimd.collective_compute(
                "AllReduce",
                mybir.AluOpType.max,
                replica_groups=[list(range(N_CORES))],
                ins=[mx_in.opt()],
                outs=[mx_out.opt()],
            )
            gmax_row = constp.tile([1, 16], F32, tag="gmax_row")
            nc.sync.dma_start(out=gmax_row[:, :], in_=mx_out[:, :])
            gmax = constp.tile([128, 1], F32, tag="gmax")
            nc.gpsimd.partition_broadcast(gmax[:, :], gmax_row[0:1, 0:1])
            # s = gmax/127 and 1/s via reciprocal (DVE has no divide); the
            # <=2ulp drift is far inside the error budget.
            s_col = constp.tile([128, 1], F32, tag="s_col")
            nc.vector.tensor_scalar(
                out=s_col[:, :], in0=gmax[:, :],
                scalar1=float(np.float32(1.0) / np.float32(127.0)),
                scalar2=None, op0=mybir.AluOpType.mult,
            )
            rgmax = constp.tile([128, 1], F32, tag="rgmax")
            nc.vector.reciprocal(rgmax[:, :], gmax[:, :])
            inv_s = constp.tile([128, 1], F32, tag="inv_s")
            nc.vector.tensor_scalar(
                out=inv_s[:, :], in0=rgmax[:, :], scalar1=127.0, scalar2=None,
                op0=mybir.AluOpType.mult,
            )

            # ---- phase C: scores + incremental argmin ----------------------
            kminbig = workp.tile([128, NT * NC], F32, tag="kminbig", bufs=1)
            for j in range(NCHUNK):
                ps_xct = psp.tile([128, TOK], F32, tag="ps")
                for mcol in range(4):
                    p = 4 * j + mcol
                    xt_t = xtp.tile([128, 2 * TOK], BF16, tag="xt")
                    nc.sync.dma_start(out=xt_t[:, :],
                                      in_=xhl.ap()[128 * p:128 * (p + 1), :])
                    passes = [(cbdh_sb, 0), (cbdh_sb, TOK), (cbdl_sb, 0)]
                    for i, (cb, off) in enumerate(passes):
                        nc.tensor.matmul(
                            ps_xct[32 * mcol:32 * (mcol + 1), :],
                            lhsT=cb[:, 32 * p:32 * (p + 1)],
                            rhs=xt_t[:, off:off + TOK],
                            start=(i == 0), stop=(i == 2),
                            tile_position=(0, 32 * mcol),
                        )
                xct_sb = xctp.tile([128, TOK], F32, tag="xct")
                nc.vector.tensor_copy(out=xct_sb[:, :], in_=ps_xct[:, :])
                ps_tr4 = psp.tile([128, TOK], F32, tag="ps")
                for t in range(NT):
                    nc.tensor.transpose(
                        ps_tr4[:, 128 * t:128 * (t + 1)],
                        xct_sb[:, 128 * t:128 * (t + 1)],
                        identf[:, :],
                    )
                # --- batched score + first-index argmin (4 token tiles) ---
                ssl = workp.tile([128, TOK], F32, tag="ssl")
                nc.vector.scalar_tensor_tensor(
                    out=ssl[:, :].rearrange("p (t f) -> p t f", f=128),
                    in0=ps_tr4[:, :].rearrange("p (t f) -> p t f", f=128),
                    in1=c2_sb[:, 128 * j:128 * (j + 1)].unsqueeze(1)
                        .broadcast_to([128, NT, 128]),
                    scalar=-2.0,
                    op0=mybir.AluOpType.mult,
                    op1=mybir.AluOpType.add,
                )
                ssl4 = ssl[:, :].rearrange("p (t c k) -> p t c k", k=K, c=8)
                m32 = workp.tile([128, NT * 8], F32, tag="m32")
                nc.vector.tensor_reduce(
                    out=m32[:, :], in_=ssl4, axis=mybir.AxisListType.X,
                    op=mybir.AluOpType.min,
                )
                eq = workp.tile([128, TOK], F32, tag="eq")
                nc.vector.tensor_tensor(
                    out=eq[:, :].rearrange("p (t c k) -> p t c k", k=K, c=8),
                    in0=ssl4,
                    in1=m32[:, :].rearrange("p (t c) -> p t c", c=8)
                        .unsqueeze(3).broadcast_to([128, NT, 8, K]),
                    op=mybir.AluOpType.is_equal,
                )
                cand = workp.tile([128, TOK], F32, tag="cand")
                nc.vector.scalar_tensor_tensor(
                    out=cand[:, :].rearrange("p (t f) -> p t f", f=128),
                    in0=eq[:, :].rearrange("p (t f) -> p t f", f=128),
                    in1=iota_sb[:, :].unsqueeze(1)
                        .broadcast_to([128, NT, 128]),
                    scalar=-BIG,
                    op0=mybir.AluOpType.mult, op1=mybir.AluOpType.add,
                )
                nc.vector.tensor_reduce(
                    out=kminbig[:, :].rearrange(
                        "p (t c) -> p t c", c=NC)[:, :, 8 * j:8 * (j + 1)],
                    in_=cand[:, :].rearrange("p (t c k) -> p t c k", k=K, c=8),
                    axis=mybir.AxisListType.X, op=mybir.AluOpType.min,
                )

            # ---- phase 2: AllGather the indices (64KB/core) ----------------
            kt_sb = constp.tile([NC, TOK], BF16, tag="kt_sb")
            for t in range(NT):
                kminb = workp.tile([128, NC], BF16, tag="kminb")
                nc.vector.tensor_scalar_add(
                    kminb[:, :], kminbig[:, NC * t:NC * (t + 1)], BIG)
                # transpose kmin [n, c] -> [c, n] (values 0..15, exact bf16)
                ps_kt = psp.tile([NC, 128], BF16, tag="ps")
                nc.tensor.transpose(ps_kt[:, :], kminb[:, :], identb[:, :])
                nc.vector.tensor_copy(
                    out=kt_sb[:, 128 * t:128 * (t + 1)], in_=ps_kt[:, :])
            nc.sync.dma_start(out=kt_bounce[:, :], in_=kt_sb[:, :])
            nc.gpsimd.collective_compute(
                "AllGather",
                mybir.AluOpType.bypass,
                replica_groups=[list(range(N_CORES))],
                ins=[kt_bounce.opt()],
                outs=[kt_all.opt()],
            )

            # ---- phase 5: quantize-dequantize lut -> bf16 ------------------
            lutdq = lutp.tile([128, NCHUNK * OSH], BF16, tag="lutdq")
            for j in range(NCHUNK):
                qm = wp.tile([128, OSH], F32, tag="qm")
                nc.vector.tensor_scalar(
                    out=qm[:, :], in0=lut_sb[:, OSH * j:OSH * (j + 1)],
                    scalar1=inv_s[:, 0:1], scalar2=MAGIC,
                    op0=mybir.AluOpType.mult, op1=mybir.AluOpType.add,
                )
                nc.vector.tensor_scalar(
                    out=lutdq[:, OSH * j:OSH * (j + 1)], in0=qm[:, :],
                    scalar1=MAGIC, scalar2=s_col[:, 0:1],
                    op0=mybir.AluOpType.subtract, op1=mybir.AluOpType.mult,
                )

            bias_sb = constp.tile([128, OSH], F32, tag="bias_sb")
            nc.sync.dma_start(out=bias_sb[:, :], in_=biasr.ap()[:, :])
            kcol_sb = constp.tile([128, 1], F32, tag="kcol_sb")
            nc.sync.dma_start(out=kcol_sb[:, :], in_=kcol.ap()[:, :])

            # ---- phase 6: expand one-hots + gather matmul ------------------
            for r in range(N_CORES):
                stg_tiles = []
                for j in range(NCHUNK):
                    # replicate each codebook's index row onto its 16
                    # k-partitions straight out of DRAM (step-0 broadcast)
                    rep = strepp.tile([128, TOK], BF16, tag="strep")
                    nc.gpsimd.dma_start(
                        out=rep[:, :],
                        in_=kt_all[NC * r + 8 * j:NC * r + 8 * (j + 1), :]
                            .unsqueeze(1).broadcast_to([8, K, TOK]),
                    )
                    stg_t = stgp.tile([128, TOK], BF16, tag="stg")
                    nc.vector.tensor_scalar(
                        out=stg_t[:, :], in0=rep[:, :],
                        scalar1=kcol_sb[:, 0:1], scalar2=None,
                        op0=mybir.AluOpType.is_equal,
                    )
                    stg_tiles.append(stg_t)
                o_sb = outp.tile([128, NT * OSH], BF16, tag="outsb")
                for t in range(NT):
                    ps_o = psp.tile([128, OSH], F32, tag="ps")
                    for j in range(NCHUNK):
                        nc.tensor.matmul(
                            ps_o[:, :],
                            lhsT=stg_tiles[j][:, 128 * t:128 * (t + 1)],
                            rhs=lutdq[:, OSH * j:OSH * (j + 1)],
                            start=(j == 0), stop=(j == NCHUNK - 1),
                        )
                    nc.vector.tensor_tensor(
                        out=o_sb[:, OSH * t:OSH * (t + 1)], in0=ps_o[:, :],
                        in1=bias_sb[:, :],
                        op=mybir.AluOpType.add,
                    )
                nc.scalar.dma_start(
                    out=out.ap()[TOK * r:TOK * (r + 1), :]
                        .rearrange("(t p) o -> p t o", p=128),
                    in_=o_sb[:, :].rearrange("p (t o) -> p t o", o=OSH),
                )

    nc.compile()
    return nc


def _prep_inputs(x, centroids, weight, bias):
    import ml_dtypes

    x = np.ascontiguousarray(np.asarray(x, dtype=np.float32)).reshape(BN, IN_F)
    cent = np.asarray(centroids, dtype=np.float32)
    w = np.asarray(weight, dtype=np.float32)
    bias = np.asarray(bias, dtype=np.float32)

    c2 = (cent ** 2).sum(axis=-1).reshape(CK)  # [1024] flat (c,k)
    c2r = np.ascontiguousarray(np.broadcast_to(c2, (128, CK)))
    iota = np.tile(np.arange(K, dtype=np.float32), 8)
    iotar = np.ascontiguousarray(np.broadcast_to(iota, (128, 128)))
    cbd = np.zeros((128, CK), np.float32)
    for p in range(NPAIR):
        cbd[0:SUBV, 32 * p:32 * p + K] = cent[2 * p].T
        cbd[SUBV:128, 32 * p + K:32 * p + 2 * K] = cent[2 * p + 1].T
    cbd_h = cbd.astype(ml_dtypes.bfloat16)
    cbd_l = (cbd - cbd_h.astype(np.float32)).astype(ml_dtypes.bfloat16)
    kcol = np.ascontiguousarray(
        (np.arange(128, dtype=np.float32) % K).reshape(128, 1))

    in_maps = []
    for r in range(N_CORES):
        xT_r = np.ascontiguousarray(x[TOK * r:TOK * (r + 1)].T)
        xh_r = xT_r.astype(ml_dtypes.bfloat16)
        xl_r = (xT_r - xh_r.astype(np.float32)).astype(ml_dtypes.bfloat16)
        xhl_r = np.ascontiguousarray(np.concatenate([xh_r, xl_r], axis=1))
        w_r = np.ascontiguousarray(w[:, :, OSH * r:OSH * (r + 1)]).reshape(
            IN_F, OSH)
        w_h = w_r.astype(ml_dtypes.bfloat16)
        w_l = (w_r - w_h.astype(np.float32)).astype(ml_dtypes.bfloat16)
        whl_r = np.ascontiguousarray(np.concatenate([w_h, w_l], axis=1))
        bias_r = np.ascontiguousarray(
            np.broadcast_to(bias[OSH * r:OSH * (r + 1)], (128, OSH)))
        in_maps.append({
            "xhl": xhl_r, "whl": whl_r, "cbd_h": cbd_h, "cbd_l": cbd_l,
            "c2r": c2r, "iotar": iotar, "biasr": bias_r, "kcol": kcol,
        })
    return in_maps


def kernel(x, centroids, weight, inverse_temperature_logit, bias,
           **_unused) -> np.ndarray:
    if "nc" not in _CACHE:
        _CACHE["nc"] = _build()
    nc = _CACHE["nc"]
    in_maps = _prep_inputs(x, centroids, weight, bias)
    res = run_bass_kernel_spmd(nc, in_maps, core_ids=list(range(N_CORES)))
    out = np.concatenate(
        [res.results[r]["out"].astype(np.float32) for r in range(N_CORES)],
        axis=1)
    return out.reshape(2, BN // 2, OUT_F)


# revision 16
# speedup vs baseline: 1.3268x; 1.0458x over previous
"""AMMLinear (VQ codebook) forward on 8 TRN2 NeuronCores.

The straight-through estimator makes the forward VALUE exactly
    out[n, o] = sum_c lut_dq[c, argmin_k dist(x_cn, cent_ck), o] + bias[o]
with lut = centroids @ weight (per codebook) and lut_dq a global-scale int8
quantize-dequantize of lut.  The softmax/attention path only shapes gradients.

Sharding: tokens (BN=4096 -> 512/core) for the score/argmin phase, output
features (4096 -> 512/core) for the lut/gather phase.  One AllGather moves the
bf16 argmin indices (64KB/core), one AllReduce-max the quantization scale.
Every core then expands all 4096 tokens' one-hot codes locally (replication
DMA + is_equal) and computes its o-shard of the gather matmul.
Output is assembled host-side by concatenating the per-core o-shards.

Numerics: scores and lut are computed as 3-pass bf16 hi/lo matmuls
(a*b ~ ah*bh + ah*bl + al*bh, fp32 PSUM accumulate, ~2^-18/product), the
gather matmul in bf16 (exact one-hots, bf16-rounded lut_dq), output in bf16.
Measured end-to-end rel err ~5e-3 against the fp32 reference.
"""

import numpy as np

import concourse.bass as bass
import concourse.mybir as mybir
import concourse.tile as tile
import concourse.bass_isa as bass_isa
from concourse import bacc
from concourse.bass_utils import run_bass_kernel_spmd
from concourse.masks import make_identity

F32 = mybir.dt.float32
BF16 = mybir.dt.bfloat16

N_CORES = 8
NC, K, IN_F, OUT_F = 64, 16, 4096, 4096
SUBV = IN_F // NC          # 64
BN = 4096                  # 2*2048 tokens
TOK = BN // N_CORES        # 512 tokens per core
NT = TOK // 128            # 4 token tiles per core
NPAIR = NC // 2            # 32 codebook pairs
CK = NC * K                # 1024 (codebook,centroid) flat index
NCHUNK = CK // 128         # 8 contraction chunks
OSH = OUT_F // N_CORES     # 512 out features per core
MAGIC = 12582912.0         # 1.5 * 2**23: fp32 round-to-nearest-even trick
BIG = 1024.0

_CACHE = {}


def _build():
    nc = bacc.Bacc("TRN2", target_bir_lowering=False, debug=False,
                   num_devices=N_CORES)

    # xhl[:, 0:TOK] = bf16 hi of x^T shard, [:, TOK:2*TOK] = bf16 lo
    xhl = nc.declare_dram_parameter("xhl", [IN_F, 2 * TOK], BF16,
                                    isOutput=False)
    # whl[:, 0:OSH] = bf16 hi of weight o-shard, [:, OSH:2*OSH] = lo
    whl = nc.declare_dram_parameter("whl", [IN_F, 2 * OSH], BF16,
                                    isOutput=False)
    cbd_h = nc.declare_dram_parameter("cbd_h", [128, CK], BF16, isOutput=False)
    cbd_l = nc.declare_dram_parameter("cbd_l", [128, CK], BF16, isOutput=False)
    c2r = nc.declare_dram_parameter("c2r", [128, CK], F32, isOutput=False)
    iotar = nc.declare_dram_parameter("iotar", [128, 128], F32, isOutput=False)
    biasr = nc.declare_dram_parameter("biasr", [128, OSH], F32, isOutput=False)
    kcol = nc.declare_dram_parameter("kcol", [128, 1], F32, isOutput=False)
    out = nc.declare_dram_parameter("out", [BN, OSH], BF16, isOutput=True)

    with tile.TileContext(nc) as tc:
        with (
            tc.tile_pool(name="consts", bufs=1) as constp,
            tc.tile_pool(name="xt", bufs=6) as xtp,
            tc.tile_pool(name="wt", bufs=6) as wp,
            tc.tile_pool(name="xct", bufs=3) as xctp,
            tc.tile_pool(name="work", bufs=2) as workp,
            tc.tile_pool(name="stg", bufs=16) as stgp,
            tc.tile_pool(name="strep", bufs=8) as strepp,
            tc.tile_pool(name="half", bufs=32) as halfp,
            tc.tile_pool(name="lut", bufs=1) as lutp,
            tc.tile_pool(name="outs", bufs=2) as outp,
            tc.tile_pool(name="ps", bufs=8, space="PSUM") as psp,
            tc.tile_pool(name="dram", bufs=1, space="DRAM") as dramp,
        ):
            # ---- constants -------------------------------------------------
            cbdh_sb = constp.tile([128, CK], BF16, tag="cbdh_sb")
            nc.sync.dma_start(out=cbdh_sb[:, :], in_=cbd_h.ap()[:, :])
            cbdl_sb = constp.tile([128, CK], BF16, tag="cbdl_sb")
            nc.sync.dma_start(out=cbdl_sb[:, :], in_=cbd_l.ap()[:, :])
            c2_sb = constp.tile([128, CK], F32, tag="c2_sb")
            nc.sync.dma_start(out=c2_sb[:, :], in_=c2r.ap()[:, :])
            iota_sb = constp.tile([128, 128], F32, tag="iota_sb")
            nc.sync.dma_start(out=iota_sb[:, :], in_=iotar.ap()[:, :])
            identb = constp.tile([128, 128], BF16, tag="identb")
            make_identity(nc, identb[:, :])
            identf = constp.tile([128, 128], F32, tag="identf")
            make_identity(nc, identf[:, :])

            # DRAM scratch for collectives
            kt_bounce1 = dramp.tile([NC // 2, TOK], BF16, tag="kt_bounce1")
            kt_bounce2 = dramp.tile([NC // 2, TOK], BF16, tag="kt_bounce2")
            kt_all1 = dramp.tile([N_CORES * NC // 2, TOK], BF16, tag="kt_all1")
            kt_all2 = dramp.tile([N_CORES * NC // 2, TOK], BF16, tag="kt_all2")
            mx_in = dramp.tile([1, 16], F32, tag="mx_in")
            mx_out = dramp.tile([1, 16], F32, tag="mx_out")

            # ---- phase A: lut = centroids @ weight (3-pass bf16 hi/lo) ----
            # Runs first so the absmax -> AllReduce(max) chain is issued as
            # early as possible (the collective queue is FIFO: the first
            # collective blocks the second until it completes).
            lut_sb = lutp.tile([128, NCHUNK * OSH], F32, tag="lut")
            mx8 = constp.tile([128, NCHUNK], F32, tag="mx8")
            for j in range(NCHUNK):
                ps_lut = psp.tile([128, OSH], F32, tag="ps")
                for mcol in range(4):
                    p = 4 * j + mcol
                    w_t = wp.tile([128, 2 * OSH], BF16, tag="wt")
                    nc.scalar.dma_start(
                        out=w_t[:, :],
                        in_=whl.ap()[128 * p:128 * (p + 1), :])
                    passes = [(cbdh_sb, 0), (cbdh_sb, OSH), (cbdl_sb, 0)]
                    for i, (cb, off) in enumerate(passes):
                        nc.tensor.matmul(
                            ps_lut[32 * mcol:32 * (mcol + 1), :],
                            lhsT=cb[:, 32 * p:32 * (p + 1)],
                            rhs=w_t[:, off:off + OSH],
                            start=(i == 0), stop=(i == 2),
                            tile_position=(0, 32 * mcol),
                        )
                nc.vector.tensor_copy(
                    out=lut_sb[:, OSH * j:OSH * (j + 1)], in_=ps_lut[:, :])
                nc.vector.tensor_reduce(
                    out=mx8[:, j:j + 1],
                    in_=lut_sb[:, OSH * j:OSH * (j + 1)],
                    axis=mybir.AxisListType.X, op=mybir.AluOpType.max,
                    apply_absolute_value=True,
                )

            # ---- phase B: global absmax -> AllReduce(max) -> scale ---------
            mxl = constp.tile([128, 1], F32, tag="mxl")
            nc.vector.tensor_reduce(
                out=mxl[:, :], in_=mx8[:, :], axis=mybir.AxisListType.X,
                op=mybir.AluOpType.max,
            )
            mxp = constp.tile([128, 1], F32, tag="mxp")
            nc.gpsimd.partition_all_reduce(
                mxp[:, :], mxl[:, :], channels=128,
                reduce_op=bass_isa.ReduceOp.max,
            )
            zrow = constp.tile([1, 16], F32, tag="zrow")
            nc.vector.memset(zrow[:, :], 0.0)
            nc.vector.tensor_copy(out=zrow[0:1, 0:1], in_=mxp[0:1, 0:1])
            nc.sync.dma_start(out=mx_in[:, :], in_=zrow[:, :])
            nc.gpsimd.collective_compute(
                "AllReduce",
                mybir.AluOpType.max,
                replica_groups=[list(range(N_CORES))],
                ins=[mx_in.opt()],
                outs=[mx_out.opt()],
            )
            gmax_row = constp.tile([1, 16], F32, tag="gmax_row")
            nc.sync.dma_start(out=gmax_row[:, :], in_=mx_out[:, :])
            gmax = constp.tile([128, 1], F32, tag="gmax")
            nc.gpsimd.partition_broadcast(gmax[:, :], gmax_row[0:1, 0:1])
            # s = gmax/127 and 1/s via reciprocal (DVE has no divide); the
            # <=2ulp drift is far inside the error budget.
            s_col = constp.tile([128, 1], F32, tag="s_col")
            nc.vector.tensor_scalar(
                out=s_col[:, :], in0=gmax[:, :],
                scalar1=float(np.float32(1.0) / np.float32(127.0)),
                scalar2=None, op0=mybir.AluOpType.mult,
            )
            rgmax = constp.tile([128, 1], F32, tag="rgmax")
            nc.vector.reciprocal(rgmax[:, :], gmax[:, :])
            inv_s = constp.tile([128, 1], F32, tag="inv_s")
            nc.vector.tensor_scalar(
                out=inv_s[:, :], in0=rgmax[:, :], scalar1=127.0, scalar2=None,
                op0=mybir.AluOpType.mult,
            )

            # ---- phase C: scores + incremental argmin ----------------------
            kminbig = workp.tile([128, NT * NC], F32, tag="kminbig", bufs=1)
            for j in range(NCHUNK):
                ps_xct = psp.tile([128, TOK], F32, tag="ps")
                for mcol in range(4):
                    p = 4 * j + mcol
                    xt_t = xtp.tile([128, 2 * TOK], BF16, tag="xt")
                    nc.sync.dma_start(out=xt_t[:, :],
                                      in_=xhl.ap()[128 * p:128 * (p + 1), :])
                    passes = [(cbdh_sb, 0), (cbdh_sb, TOK), (cbdl_sb, 0)]
                    for i, (cb, off) in enumerate(passes):
                        nc.tensor.matmul(
                            ps_xct[32 * mcol:32 * (mcol + 1), :],
                            lhsT=cb[:, 32 * p:32 * (p + 1)],
                            rhs=xt_t[:, off:off + TOK],
                            start=(i == 0), stop=(i == 2),
                            tile_position=(0, 32 * mcol),
                        )
                xct_sb = xctp.tile([128, TOK], F32, tag="xct")
                nc.vector.tensor_copy(out=xct_sb[:, :], in_=ps_xct[:, :])
                ps_tr4 = psp.tile([128, TOK], F32, tag="ps")
                for t in range(NT):
                    nc.tensor.transpose(
                        ps_tr4[:, 128 * t:128 * (t + 1)],
                        xct_sb[:, 128 * t:128 * (t + 1)],
                        identf[:, :],
                    )
                # --- batched score + first-index argmin (4 token tiles) ---
                ssl = workp.tile([128, TOK], F32, tag="ssl")
                nc.vector.scalar_tensor_tensor(
                    out=ssl[:, :].rearrange("p (t f) -> p t f", f=128),
                    in0=ps_tr4[:, :].rearrange("p (t f) -> p t f", f=128),
                    in1=c2_sb[:, 128 * j:128 * (j + 1)].unsqueeze(1)
                        .broadcast_to([128, NT, 128]),
                    scalar=-2.0,
                    op0=mybir.AluOpType.mult,
                    op1=mybir.AluOpType.add,
                )
                ssl4 = ssl[:, :].rearrange("p (t c k) -> p t c k", k=K, c=8)
                m32 = workp.tile([128, NT * 8], F32, tag="m32")
                nc.vector.tensor_reduce(
                    out=m32[:, :], in_=ssl4, axis=mybir.AxisListType.X,
                    op=mybir.AluOpType.min,
                )
                eq = workp.tile([128, TOK], F32, tag="eq")
                nc.vector.tensor_tensor(
                    out=eq[:, :].rearrange("p (t c k) -> p t c k", k=K, c=8),
                    in0=ssl4,
                    in1=m32[:, :].rearrange("p (t c) -> p t c", c=8)
                        .unsqueeze(3).broadcast_to([128, NT, 8, K]),
                    op=mybir.AluOpType.is_equal,
                )
                cand = workp.tile([128, TOK], F32, tag="cand")
                nc.vector.scalar_tensor_tensor(
                    out=cand[:, :].rearrange("p (t f) -> p t f", f=128),
                    in0=eq[:, :].rearrange("p (t f) -> p t f", f=128),
                    in1=iota_sb[:, :].unsqueeze(1)
                        .broadcast_to([128, NT, 128]),
                    scalar=-BIG,
                    op0=mybir.AluOpType.mult, op1=mybir.AluOpType.add,
                )
                nc.vector.tensor_reduce(
                    out=kminbig[:, :].rearrange(
                        "p (t c) -> p t c", c=NC)[:, :, 8 * j:8 * (j + 1)],
                    in_=cand[:, :].rearrange("p (t c k) -> p t c k", k=K, c=8),
                    axis=mybir.AxisListType.X, op=mybir.AluOpType.min,
                )
                # After chunks 0-3 / 4-7 complete, ship that half of the
                # indices: kmin [n, c-half] -> bf16 -> transpose -> AllGather.
                # Collective order (AR, AG1, AG2) matches input readiness so
                # the FIFO collective queue never head-of-line blocks.
                if j == 3 or j == NCHUNK - 1:
                    half = 0 if j == 3 else 1
                    c0 = (NC // 2) * half
                    kth = constp.tile([NC // 2, TOK], BF16, tag=f"kt_sb{half}",
                                      name=f"kt_sb{half}")
                    for t in range(NT):
                        kminb = workp.tile([128, NC // 2], BF16, tag="kminb")
                        nc.vector.tensor_scalar_add(
                            kminb[:, :],
                            kminbig[:, NC * t + c0:NC * t + c0 + NC // 2],
                            BIG)
                        ps_kt = psp.tile([NC // 2, 128], BF16, tag="ps")
                        nc.tensor.transpose(ps_kt[:, :], kminb[:, :],
                                            identb[:, :])
                        nc.vector.tensor_copy(
                            out=kth[:, 128 * t:128 * (t + 1)], in_=ps_kt[:, :])
                    bnc = kt_bounce1 if half == 0 else kt_bounce2
                    gat = kt_all1 if half == 0 else kt_all2
                    nc.sync.dma_start(out=bnc[:, :], in_=kth[:, :])
                    nc.gpsimd.collective_compute(
                        "AllGather",
                        mybir.AluOpType.bypass,
                        replica_groups=[list(range(N_CORES))],
                        ins=[bnc.opt()],
                        outs=[gat.opt()],
                    )

            # ---- phase 5: quantize-dequantize lut -> bf16 ------------------
            lutdq = lutp.tile([128, NCHUNK * OSH], BF16, tag="lutdq")
            for j in range(NCHUNK):
                qm = wp.tile([128, OSH], F32, tag="qm")
                nc.vector.tensor_scalar(
                    out=qm[:, :], in0=lut_sb[:, OSH * j:OSH * (j + 1)],
                    scalar1=inv_s[:, 0:1], scalar2=MAGIC,
                    op0=mybir.AluOpType.mult, op1=mybir.AluOpType.add,
                )
                nc.vector.tensor_scalar(
                    out=lutdq[:, OSH * j:OSH * (j + 1)], in0=qm[:, :],
                    scalar1=MAGIC, scalar2=s_col[:, 0:1],
                    op0=mybir.AluOpType.subtract, op1=mybir.AluOpType.mult,
                )

            bias_sb = constp.tile([128, OSH], F32, tag="bias_sb")
            nc.sync.dma_start(out=bias_sb[:, :], in_=biasr.ap()[:, :])
            kcol_sb = constp.tile([128, 1], F32, tag="kcol_sb")
            nc.sync.dma_start(out=kcol_sb[:, :], in_=kcol.ap()[:, :])

            # ---- phase 6: expand one-hots + gather matmul ------------------
            # Two half-contractions: chunks 0-3 right after AG1 (overlapping
            # AG2), chunks 4-7 after AG2; bias folded into the first half.
            half_tiles = {}
            for half in range(2):
                gat = kt_all1 if half == 0 else kt_all2
                for r in range(N_CORES):
                    stg_tiles = []
                    for jj in range(NCHUNK // 2):
                        rep = strepp.tile([128, TOK], BF16, tag="strep")
                        nc.gpsimd.dma_start(
                            out=rep[:, :],
                            in_=gat[32 * r + 8 * jj:32 * r + 8 * (jj + 1), :]
                                .unsqueeze(1).broadcast_to([8, K, TOK]),
                        )
                        stg_t = stgp.tile([128, TOK], BF16, tag="stg")
                        nc.vector.tensor_scalar(
                            out=stg_t[:, :], in0=rep[:, :],
                            scalar1=kcol_sb[:, 0:1], scalar2=None,
                            op0=mybir.AluOpType.is_equal,
                        )
                        stg_tiles.append(stg_t)
                    o_sb = None
                    if half == 1:
                        o_sb = outp.tile([128, NT * OSH], BF16, tag="outsb")
                    for t in range(NT):
                        ps_o = psp.tile([128, OSH], F32, tag="ps")
                        for jj in range(NCHUNK // 2):
                            jglob = 4 * half + jj
                            nc.tensor.matmul(
                                ps_o[:, :],
                                lhsT=stg_tiles[jj][:, 128 * t:128 * (t + 1)],
                                rhs=lutdq[:, OSH * jglob:OSH * (jglob + 1)],
                                start=(jj == 0), stop=(jj == NCHUNK // 2 - 1),
                            )
                        if half == 0:
                            hs = halfp.tile([128, OSH], BF16, tag="hsum",
                                           name=f"hs{r}_{t}")
                            nc.vector.tensor_tensor(
                                out=hs[:, :], in0=ps_o[:, :], in1=bias_sb[:, :],
                                op=mybir.AluOpType.add,
                            )
                            half_tiles[(r, t)] = hs
                        else:
                            nc.vector.tensor_tensor(
                                out=o_sb[:, OSH * t:OSH * (t + 1)],
                                in0=ps_o[:, :], in1=half_tiles[(r, t)][:, :],
                                op=mybir.AluOpType.add,
                            )
                    if half == 1:
                        nc.scalar.dma_start(
                            out=out.ap()[TOK * r:TOK * (r + 1), :]
                                .rearrange("(t p) o -> p t o", p=128),
                            in_=o_sb[:, :].rearrange("p (t o) -> p t o", o=OSH),
                        )

    nc.compile()
    return nc


def _prep_inputs(x, centroids, weight, bias):
    import ml_dtypes

    x = np.ascontiguousarray(np.asarray(x, dtype=np.float32)).reshape(BN, IN_F)
    cent = np.asarray(centroids, dtype=np.float32)
    w = np.asarray(weight, dtype=np.float32)
    bias = np.asarray(bias, dtype=np.float32)

    c2 = (cent ** 2).sum(axis=-1).reshape(CK)  # [1024] flat (c,k)
    c2r = np.ascontiguousarray(np.broadcast_to(c2, (128, CK)))
    iota = np.tile(np.arange(K, dtype=np.float32), 8)
    iotar = np.ascontiguousarray(np.broadcast_to(iota, (128, 128)))
    cbd = np.zeros((128, CK), np.float32)
    for p in range(NPAIR):
        cbd[0:SUBV, 32 * p:32 * p + K] = cent[2 * p].T
        cbd[SUBV:128, 32 * p + K:32 * p + 2 * K] = cent[2 * p + 1].T
    cbd_h = cbd.astype(ml_dtypes.bfloat16)
    cbd_l = (cbd - cbd_h.astype(np.float32)).astype(ml_dtypes.bfloat16)
    kcol = np.ascontiguousarray(
        (np.arange(128, dtype=np.float32) % K).reshape(128, 1))

    in_maps = []
    for r in range(N_CORES):
        xT_r = np.ascontiguousarray(x[TOK * r:TOK * (r + 1)].T)
        xh_r = xT_r.astype(ml_dtypes.bfloat16)
        xl_r = (xT_r - xh_r.astype(np.float32)).astype(ml_dtypes.bfloat16)
        xhl_r = np.ascontiguousarray(np.concatenate([xh_r, xl_r], axis=1))
        w_r = np.ascontiguousarray(w[:, :, OSH * r:OSH * (r + 1)]).reshape(
            IN_F, OSH)
        w_h = w_r.astype(ml_dtypes.bfloat16)
        w_l = (w_r - w_h.astype(np.float32)).astype(ml_dtypes.bfloat16)
        whl_r = np.ascontiguousarray(np.concatenate([w_h, w_l], axis=1))
        bias_r = np.ascontiguousarray(
            np.broadcast_to(bias[OSH * r:OSH * (r + 1)], (128, OSH)))
        in_maps.append({
            "xhl": xhl_r, "whl": whl_r, "cbd_h": cbd_h, "cbd_l": cbd_l,
            "c2r": c2r, "iotar": iotar, "biasr": bias_r, "kcol": kcol,
        })
    return in_maps


def kernel(x, centroids, weight, inverse_temperature_logit, bias,
           **_unused) -> np.ndarray:
    if "nc" not in _CACHE:
        _CACHE["nc"] = _build()
    nc = _CACHE["nc"]
    in_maps = _prep_inputs(x, centroids, weight, bias)
    res = run_bass_kernel_spmd(nc, in_maps, core_ids=list(range(N_CORES)))
    out = np.concatenate(
        [res.results[r]["out"].astype(np.float32) for r in range(N_CORES)],
        axis=1)
    return out.reshape(2, BN // 2, OUT_F)
